# revision 4
# baseline (speedup 1.0000x reference)
"""MedianConvolution (gnn message passing) — Trainium2 Bass kernel, 8 cores. v2

Computes: h = x @ kernel; msg = h[neighbors]; out = exact midpoint median
over the K=32 neighbor axis (ranks 15,16 of the sort).

v2 changes vs baseline:
  - h stored fp16 as row-PAIRS [nrows/2, 128]: one dma_gather per neighbor
    (idx = nbr>>1 fits int16; 256B blocks) instead of the lo/hi double
    gather -> gather DMA and Pool dispatch halved.
  - parity select on-chip: res = pair[0:64] ; copy_predicated overwrites
    with pair[64:128] where (nbr&1) using a host parity mask broadcast
    over units.
  - whole sort pipeline in fp16 (2x DVE throughput); x/kernel in bf16
    (halves phase-1 HBM traffic); PSUM->SBUF copies batched 8 col-tiles
    per ACT op.

Distribution: nodes sharded across 8 cores; every core computes the full
h = x @ kernel on-device (x replicated) and gathers its shard's rows.
"""
from contextlib import ExitStack

import numpy as np

import concourse.bass as bass
import concourse.tile as tile
from concourse import bacc, bass_utils, library_config, mybir
from concourse.tile_rust import add_dep_helper

F32 = mybir.dt.float32
F16 = mybir.dt.float16
BF16 = mybir.dt.bfloat16
I16 = mybir.dt.int16
U8 = mybir.dt.uint8
P = 128
U = 64  # units
K = 32  # neighbors
FEAT = 256
N_NODES = 50000
NUM_CORES = 8
CHUNK = 256  # shard nodes per chunk
NET_BUFS = 3
POOL_OPS = frozenset()

# Batcher odd-even mergesort(16) stages (verified via the 0-1 principle).
SORT16_STAGES = [
    dict(f=2, i=(0, 8, 1), r=(0, 1, 1), d=1, cp=[]),
    dict(f=4, i=(0, 4, 1), r=(0, 2, 1), d=2, cp=[]),
    dict(f=4, i=(0, 4, 1), r=(1, 2, 1), d=1, cp=[(0, 16, 4), (3, 16, 4)]),
    dict(f=8, i=(0, 2, 1), r=(0, 4, 1), d=4, cp=[]),
    dict(f=8, i=(0, 2, 1), r=(2, 4, 1), d=2,
         cp=[(0, 16, 8), (1, 16, 8), (6, 16, 8), (7, 16, 8)]),
    dict(f=8, i=(0, 2, 1), r=(1, 6, 2), d=1, cp=[(0, 16, 8), (7, 16, 8)]),
    dict(f=16, i=(0, 1, 1), r=(0, 8, 1), d=8, cp=[]),
    dict(f=16, i=(0, 1, 1), r=(4, 8, 1), d=4, cp=[(0, 4, 1), (12, 16, 1)]),
    dict(f=4, i=(0, 3, 1), r=(2, 4, 1), d=2, cp=[(0, 2, 1), (14, 16, 1)]),
    dict(f=2, i=(0, 7, 1), r=(1, 2, 1), d=1, cp=[(0, 16, 15)]),
]


def build_kernel(nrows, shard_nodes, C, num_cores=NUM_CORES, gemm_super=2048,
                 net_bufs=NET_BUFS, pool_ops=POOL_OPS, pool_every=0, lookahead=1,
                 pool_cols=0, srt_sets=2):
    assert nrows % 2 == 0
    HALF = nrows // 2
    NCHUNK = shard_nodes // C
    assert NCHUNK * C == shard_nodes
    B = C // P
    NIDX = C * K
    IDXCOLS = NIDX // 16
    BU = B * U          # sort plane width (fp16 elems)
    KB = K * B          # gather planes per chunk

    nc = bacc.Bacc(
        "TRN2",
        target_bir_lowering=False,
        debug=False,
        num_devices=num_cores,
    )

    xT = nc.dram_tensor("xT", [FEAT, nrows], F16, kind="ExternalInput").ap()
    wk = nc.dram_tensor("wk", [FEAT, U], F16, kind="ExternalInput").ap()
    idx = nc.dram_tensor("idx", [NCHUNK, P, IDXCOLS], I16, kind="ExternalInput").ap()
    par = nc.dram_tensor("par", [NCHUNK, P, KB], I16, kind="ExternalInput").ap()
    out = nc.dram_tensor("out", [NCHUNK, P, B * U], F32, kind="ExternalOutput").ap()
    # h rows fp16; gathered as row-pairs [HALF, 128]
    h = nc.dram_tensor("h", [nrows, U], F16, kind="Internal").ap()
    h_pairs = h.rearrange("(hh two) u -> hh (two u)", two=2)

    with tile.TileContext(nc) as tc:
        with ExitStack() as ctx:
            # ---------------- phase 1: GEMM ----------------
            ctx1 = ctx.enter_context(ExitStack())
            g_x = ctx1.enter_context(tc.tile_pool(name="g_x", bufs=2))
            g_w = ctx1.enter_context(tc.tile_pool(name="g_w", bufs=1))
            g_h = ctx1.enter_context(tc.tile_pool(name="g_h", bufs=2))
            g_ps = ctx1.enter_context(tc.tile_pool(name="g_ps", bufs=2, space="PSUM"))

            wkt = g_w.tile([P, 2 * U], F16)
            nc.sync.dma_start(wkt[:, 0:U], wk[0:P, :])
            nc.sync.dma_start(wkt[:, U : 2 * U], wk[P : 2 * P, :])

            h_writes = []
            S = gemm_super
            n_super = (nrows + S - 1) // S
            TPG = 8  # col-tiles per PSUM bank (8*64 = 512 fp32 = 2KB)
            for s in range(n_super):
                n0 = s * S
                ncnt = min(S, nrows - n0)
                ntiles = (ncnt + P - 1) // P
                xt0 = g_x.tile([P, S], F16, tag="xt0")
                xt1 = g_x.tile([P, S], F16, tag="xt1")
                nc.sync.dma_start(xt0[:, 0:ncnt], xT[0:P, n0 : n0 + ncnt])
                nc.sync.dma_start(xt1[:, 0:ncnt], xT[P : 2 * P, n0 : n0 + ncnt])
                hb = g_h.tile([P, (S // P) * U], F16, tag="hb")
                for tg in range(0, ntiles, TPG):
                    tn = min(TPG, ntiles - tg)
                    ps = g_ps.tile([P, TPG * U], F32)
                    for t in range(tg, tg + tn):
                        c0 = t * P
                        cw = min(P, ncnt - c0)
                        pslice = ps[0:cw, (t - tg) * U : (t - tg + 1) * U]
                        nc.tensor.matmul(
                            pslice, xt0[:, c0 : c0 + cw], wkt[:, 0:U],
                            start=True, stop=False,
                        )
                        nc.tensor.matmul(
                            pslice, xt1[:, c0 : c0 + cw], wkt[:, U : 2 * U],
                            start=False, stop=True,
                        )
                    # one batched fp32->fp16 copy per PSUM bank
                    nc.scalar.copy(
                        hb[:, tg * U : (tg + tn) * U], ps[:, 0 : tn * U]
                    )
                hb3 = hb[:].rearrange("p (t u) -> p t u", u=U)
                # write h rows [n0, n0+ncnt) (full 128-row tiles coalesced)
                lim0, lim1 = n0, n0 + ncnt
                ta = 0
                tb = (lim1 - n0) // P
                segs = []
                if tb > ta:
                    segs.append((n0, n0 + tb * P))
                if n0 + tb * P < lim1:
                    segs.append((n0 + tb * P, lim1))
                for r0, r1 in segs:
                    nt = (r1 - r0) // P
                    if nt >= 1 and (r0 - n0) % P == 0:
                        tt = (r0 - n0) // P
                        dr = h[r0:r1, :].rearrange("(o p) u -> p o u", p=P)
                        srcv = hb3[:, tt : tt + nt, :]
                    else:
                        tt = (r0 - n0) // P
                        p0 = r0 - (n0 + tt * P)
                        p1 = r1 - (n0 + tt * P)
                        dr = h[r0:r1, :].rearrange("(o p) u -> p o u", p=p1 - p0)
                        srcv = hb3[p0:p1, tt : tt + 1, :]
                    h_writes.append(nc.sync.dma_start(dr, srcv))

            # ---------------- phase 2: gather + select + median ----------------
            ctx1.close()
            g_net = ctx.enter_context(tc.tile_pool(name="g_net", bufs=net_bufs))
            g_srt = ctx.enter_context(tc.tile_pool(name="g_srt", bufs=2 * srt_sets))
            g_idx = ctx.enter_context(tc.tile_pool(name="g_idx", bufs=2))
            g_out = ctx.enter_context(tc.tile_pool(name="g_out", bufs=2))
            g_big = ctx.enter_context(tc.tile_pool(name="g_big", bufs=1))

            nc.gpsimd.load_library(library_config.mlp)
            med_all = g_big.tile([P, NCHUNK * B * U], F32, tag="medall")
            n_g = 0
            # per-call index count capped by the SWDGE ring
            KG = max(1, 1792 // C)
            kgroups = []
            k0 = 0
            while k0 < K:
                kgroups.append((k0, min(K, k0 + KG)))
                k0 += KG

            gat_tiles = {}

            def emit_tt(out, in0, in1, op):
                if pool_cols:
                    cs = out.shape[-1] - pool_cols
                    sl_d = (Ellipsis, slice(0, cs))
                    sl_p = (Ellipsis, slice(cs, out.shape[-1]))
                    nc.vector.tensor_tensor(
                        out=out[sl_d], in0=in0[sl_d], in1=in1[sl_d], op=op)
                    nc.gpsimd.tensor_tensor(
                        out=out[sl_p], in0=in0[sl_p], in1=in1[sl_p], op=op)
                else:
                    nc.vector.tensor_tensor(out=out, in0=in0, in1=in1, op=op)

            def emit_gather(c):
                nonlocal n_g
                if c >= NCHUNK or c in gat_tiles:
                    return
                ia = g_idx.tile([P, IDXCOLS], I16, tag="ia")
                nc.sync.dma_start(ia[:], idx[c])
                gat = g_net.tile([P, KB * 2 * U], F16, tag="gat")
                gat3 = gat[:].rearrange("p (j e) -> p j e", e=2 * U)
                for ka, kb in kgroups:
                    nidx = C * (kb - ka)
                    g = nc.gpsimd.dma_gather(
                        gat3[:, ka * B : kb * B, :],
                        h_pairs,
                        ia[:, ka * C // 16 : kb * C // 16],
                        nidx,
                        nidx,
                        2 * U,
                        single_packet=False,
                    )
                    if n_g == 0:
                        for w in h_writes:
                            add_dep_helper(
                                g.ins, w.ins,
                                reason="gather waits for h DRAM writes",
                            )
                    n_g += 1
                gat_tiles[c] = gat

            for c in range(lookahead):
                emit_gather(c)

            for c in range(NCHUNK):
                on_pool = pool_every and (c % pool_every == pool_every - 1)

                def VE(tag, _on_pool=on_pool):
                    return nc.gpsimd if (_on_pool or tag in pool_ops) else nc.vector

                emit_gather(c + lookahead)
                gat = gat_tiles.pop(c)
                gat3 = gat[:].rearrange("p (j e) -> p j e", e=2 * U)
                pa = g_idx.tile([P, KB], I16, tag="pa")
                nc.sync.dma_start(pa[:], par[c])
                # parity select in place: overwrite gat's E columns with O
                # where the neighbor is odd; stage 1 then reads the selected
                # (strided) columns directly.
                ra = g_srt.tile([P, K * BU], F16, tag="ra")
                rb = g_srt.tile([P, K * BU], F16, tag="rb")
                nc.vector.copy_predicated(
                    out=gat3[:, :, 0:U],
                    mask=pa[:].rearrange("p (j o) -> p j o", o=1).to_broadcast([P, KB, U]),
                    data=gat3[:, :, U : 2 * U],
                )

                # Batcher network; stage 1 reads gat (strided), writes ra;
                # stages 2+ ping-pong ra <-> rb
                gsel = gat[:].rearrange("p (hi r b e) -> p hi r b e", hi=16, r=2, e=2 * U)
                src, dst = rb, ra
                for si, sp in enumerate(SORT16_STAGES):
                    eng = VE(f"s{si}")
                    f = sp["f"]
                    ni = 16 // f
                    i_full = sp["i"] == (0, ni, 1)
                    d = sp["d"]
                    di, dr = d // f, d % f
                    r_vals = list(range(*sp["r"]))
                    if r_vals[-1] + dr >= f:
                        assert all(rv + dr >= f for rv in r_vals), sp
                        di, dr = di + 1, dr - f
                    r_sl = slice(*sp["r"])
                    hi_r = slice(sp["r"][0] + dr, sp["r"][1] + dr, sp["r"][2])
                    if i_full and di == 0:
                        if si == 0:
                            vs = None
                            vd = dst[:].rearrange(
                                "p (hi r b u) -> p hi r b u", hi=16, r=2, u=U
                            )
                            lo_s = gsel[:, :, r_sl, :, 0:U]
                            hi_s = gsel[:, :, hi_r, :, 0:U]
                            emit_tt(vd[:, :, r_sl, :, :], lo_s, hi_s,
                                    mybir.AluOpType.min)
                            emit_tt(vd[:, :, hi_r, :, :], lo_s, hi_s,
                                    mybir.AluOpType.max)
                            src, dst = dst, src
                            continue
                        vs = src[:].rearrange("p (hi r bu) -> p hi r bu", r=f, bu=BU)
                        vd = dst[:].rearrange("p (hi r bu) -> p hi r bu", r=f, bu=BU)
                        lo_s = vs[:, :, r_sl, :]
                        hi_s = vs[:, :, hi_r, :]
                        emit_tt(vd[:, :, r_sl, :], lo_s, hi_s,
                                mybir.AluOpType.min)
                        emit_tt(vd[:, :, hi_r, :], lo_s, hi_s,
                                mybir.AluOpType.max)
                    else:
                        i_sl = slice(*sp["i"])
                        hi_i = slice(sp["i"][0] + di, sp["i"][1] + di, sp["i"][2])
                        vs = src[:].rearrange(
                            "p (hh i r bu) -> p hh i r bu", hh=2, i=ni, r=f, bu=BU
                        )
                        vd = dst[:].rearrange(
                            "p (hh i r bu) -> p hh i r bu", hh=2, i=ni, r=f, bu=BU
                        )
                        lo_s = vs[:, :, i_sl, r_sl, :]
                        hi_s = vs[:, :, hi_i, hi_r, :]
                        emit_tt(vd[:, :, i_sl, r_sl, :], lo_s, hi_s,
                                mybir.AluOpType.min)
                        emit_tt(vd[:, :, hi_i, hi_r, :], lo_s, hi_s,
                                mybir.AluOpType.max)
                    vks = src[:].rearrange("p (hh kk bu) -> p hh kk bu", hh=2, kk=16)
                    vkd = dst[:].rearrange("p (hh kk bu) -> p hh kk bu", hh=2, kk=16)
                    for cpsl in sp["cp"]:
                        ks = slice(*cpsl)
                        nc.scalar.copy(vkd[:, :, ks, :], vks[:, :, ks, :])
                    src, dst = dst, src

                # anti-diagonal merge of the two sorted 16-plane halves
                vk = src[:].rearrange("p (k bu) -> p k bu", k=K)
                vo = dst[:].rearrange("p (k bu) -> p k bu", k=K)
                A = vk[:, 0:16, :]
                Brev = vk[:, 31:15:-1, :]
                emit_tt(vo[:, 0:16, :], A, Brev, mybir.AluOpType.max)
                emit_tt(vk[:, 0:16, :], A, Brev, mybir.AluOpType.min)
                # tree reductions over the 16 planes (packed fp16 TT ops):
                # low = max over min-planes (in vk), up = min over max-planes (vo)
                for buf, op in ((vk, mybir.AluOpType.max), (vo, mybir.AluOpType.min)):
                    w = 8
                    while w >= 1:
                        emit_tt(buf[:, 0:w, :], buf[:, 0:w, :],
                                buf[:, w : 2 * w, :], op)
                        w //= 2
                ms = med_all[:, c * BU : (c + 1) * BU]
                nc.vector.tensor_tensor(
                    out=ms, in0=vk[:, 0, :], in1=vo[:, 0, :], op=mybir.AluOpType.add
                )
                nc.scalar.mul(ms, ms, 0.5)
                nc.sync.dma_start(out[c], ms)

    nc.compile()
    return nc


def _prep_inputs(x, neighbors, kern, num_cores=NUM_CORES, C=CHUNK):
    import ml_dtypes

    nrows = x.shape[0]
    total = neighbors.shape[0]
    shard = (total + num_cores - 1) // num_cores
    NCHUNK = (shard + C - 1) // C
    shard_pad = NCHUNK * C
    B = C // P
    NIDX = C * K
    IDXCOLS = NIDX // 16
    KB = K * B

    xT = np.ascontiguousarray(x.T).astype(np.float16)
    wk = np.ascontiguousarray(kern).astype(np.float16)

    in_maps = []
    for core in range(num_cores):
        n0 = core * shard
        nbr = np.zeros((shard_pad, K), dtype=np.int64)
        real = min(shard, total - n0)
        nbr[:real] = neighbors[n0 : n0 + real]
        idxarr = np.empty((NCHUNK, P, IDXCOLS), dtype=np.int16)
        pararr = np.empty((NCHUNK, P, KB), dtype=np.int16)
        for c in range(NCHUNK):
            nb3 = nbr[c * C : (c + 1) * C].reshape(B, P, K)
            v = nb3.transpose(2, 0, 1).reshape(-1)  # i = ((k*B + b)*128 + p)
            pair = (v >> 1).astype(np.int16)
            # logical index i lives at [i%16, i//16]; replicated to all
            # eight 16-partition groups (Q7 core pairs read their own)
            idxarr[c] = np.tile(pair.reshape(IDXCOLS, 16).T, (P // 16, 1))
            pararr[c] = (v & 1).astype(np.int16).reshape(KB, P).T
        in_maps.append({"xT": xT, "wk": wk, "idx": idxarr, "par": pararr})
    meta = dict(shard=shard, shard_pad=shard_pad, NCHUNK=NCHUNK, C=C, total=total)
    return in_maps, meta


def _unshard_output(results, meta, num_cores=NUM_CORES):
    outs = []
    for core in range(num_cores):
        o = results[core]["out"]  # [NCHUNK, P, B*U]
        NCHUNK, _, BU_ = o.shape
        B = BU_ // U
        o = (
            o.reshape(NCHUNK, P, B, U)
            .transpose(0, 2, 1, 3)
            .reshape(meta["shard_pad"], U)
        )
        outs.append(o[: meta["shard"]])
    return np.concatenate(outs, axis=0)[: meta["total"]]


_CACHE = {}


def kernel(x, neighbors, kernel):
    """Full inputs in, full output out. Shards nodes across 8 NeuronCores."""
    x = np.asarray(x, dtype=np.float32)
    neighbors_np = np.asarray(neighbors)
    kern = np.asarray(kernel, dtype=np.float32)
    assert x.shape[1] == FEAT and kern.shape == (FEAT, U)
    assert neighbors_np.shape[1] == K

    in_maps, meta = _prep_inputs(x, neighbors_np, kern)
    key = (x.shape[0], meta["shard_pad"], meta["C"])
    if key not in _CACHE:
        _CACHE[key] = build_kernel(x.shape[0], meta["shard_pad"], meta["C"])
    nc = _CACHE[key]
    res = bass_utils.run_bass_kernel_spmd(
        nc, in_maps, core_ids=list(range(NUM_CORES))
    )
    return _unshard_output(res.results, meta)


# revision 5
# speedup vs baseline: 1.0021x; 1.0021x over previous
"""MedianConvolution (gnn message passing) — Trainium2 Bass kernel, 8 cores. v2

Computes: h = x @ kernel; msg = h[neighbors]; out = exact midpoint median
over the K=32 neighbor axis (ranks 15,16 of the sort).

v2 changes vs baseline:
  - h stored fp16 as row-PAIRS [nrows/2, 128]: one dma_gather per neighbor
    (idx = nbr>>1 fits int16; 256B blocks) instead of the lo/hi double
    gather -> gather DMA and Pool dispatch halved.
  - parity select on-chip: res = pair[0:64] ; copy_predicated overwrites
    with pair[64:128] where (nbr&1) using a host parity mask broadcast
    over units.
  - whole sort pipeline in fp16 (2x DVE throughput); x/kernel in bf16
    (halves phase-1 HBM traffic); PSUM->SBUF copies batched 8 col-tiles
    per ACT op.

Distribution: nodes sharded across 8 cores; every core computes the full
h = x @ kernel on-device (x replicated) and gathers its shard's rows.
"""
from contextlib import ExitStack

import numpy as np

import concourse.bass as bass
import concourse.tile as tile
from concourse import bacc, bass_utils, library_config, mybir
from concourse.tile_rust import add_dep_helper

F32 = mybir.dt.float32
F16 = mybir.dt.float16
BF16 = mybir.dt.bfloat16
I16 = mybir.dt.int16
U8 = mybir.dt.uint8
P = 128
U = 64  # units
K = 32  # neighbors
FEAT = 256
N_NODES = 50000
NUM_CORES = 8
CHUNK = 256  # shard nodes per chunk
NET_BUFS = 3
POOL_OPS = frozenset()

# Batcher odd-even mergesort(16) stages (verified via the 0-1 principle).
SORT16_STAGES = [
    dict(f=2, i=(0, 8, 1), r=(0, 1, 1), d=1, cp=[]),
    dict(f=4, i=(0, 4, 1), r=(0, 2, 1), d=2, cp=[]),
    dict(f=4, i=(0, 4, 1), r=(1, 2, 1), d=1, cp=[(0, 16, 4), (3, 16, 4)]),
    dict(f=8, i=(0, 2, 1), r=(0, 4, 1), d=4, cp=[]),
    dict(f=8, i=(0, 2, 1), r=(2, 4, 1), d=2,
         cp=[(0, 16, 8), (1, 16, 8), (6, 16, 8), (7, 16, 8)]),
    dict(f=8, i=(0, 2, 1), r=(1, 6, 2), d=1, cp=[(0, 16, 8), (7, 16, 8)]),
    dict(f=16, i=(0, 1, 1), r=(0, 8, 1), d=8, cp=[]),
    dict(f=16, i=(0, 1, 1), r=(4, 8, 1), d=4, cp=[(0, 4, 1), (12, 16, 1)]),
    dict(f=4, i=(0, 3, 1), r=(2, 4, 1), d=2, cp=[(0, 2, 1), (14, 16, 1)]),
    dict(f=2, i=(0, 7, 1), r=(1, 2, 1), d=1, cp=[(0, 16, 15)]),
]


def build_kernel(nrows, shard_nodes, C, num_cores=NUM_CORES, gemm_super=2048,
                 net_bufs=NET_BUFS, pool_ops=POOL_OPS, pool_every=0, lookahead=1,
                 pool_cols=0, srt_sets=2, XBUFS=3):
    assert nrows % 2 == 0
    HALF = nrows // 2
    NCHUNK = shard_nodes // C
    assert NCHUNK * C == shard_nodes
    B = C // P
    NIDX = C * K
    IDXCOLS = NIDX // 16
    BU = B * U          # sort plane width (fp16 elems)
    KB = K * B          # gather planes per chunk

    nc = bacc.Bacc(
        "TRN2",
        target_bir_lowering=False,
        debug=False,
        num_devices=num_cores,
    )

    xT = nc.dram_tensor("xT", [FEAT, nrows], F16, kind="ExternalInput").ap()
    wk = nc.dram_tensor("wk", [FEAT, U], F16, kind="ExternalInput").ap()
    idx = nc.dram_tensor("idx", [NCHUNK, P, IDXCOLS], I16, kind="ExternalInput").ap()
    par = nc.dram_tensor("par", [NCHUNK, P, KB], I16, kind="ExternalInput").ap()
    out = nc.dram_tensor("out", [NCHUNK, P, B * U], F32, kind="ExternalOutput").ap()
    # h rows fp16; gathered as row-pairs [HALF, 128]
    h = nc.dram_tensor("h", [nrows, U], F16, kind="Internal").ap()
    h_pairs = h.rearrange("(hh two) u -> hh (two u)", two=2)

    with tile.TileContext(nc) as tc:
        with ExitStack() as ctx:
            # ---------------- phase 1: GEMM ----------------
            ctx1 = ctx.enter_context(ExitStack())
            g_x = ctx1.enter_context(tc.tile_pool(name="g_x", bufs=XBUFS))
            g_w = ctx1.enter_context(tc.tile_pool(name="g_w", bufs=1))
            g_h = ctx1.enter_context(tc.tile_pool(name="g_h", bufs=2))
            g_ps = ctx1.enter_context(tc.tile_pool(name="g_ps", bufs=2, space="PSUM"))

            wkt = g_w.tile([P, 2 * U], F16)
            nc.sync.dma_start(wkt[:, 0:U], wk[0:P, :])
            nc.sync.dma_start(wkt[:, U : 2 * U], wk[P : 2 * P, :])

            h_writes = []
            S = gemm_super
            n_super = (nrows + S - 1) // S
            TPG = 8  # col-tiles per PSUM bank (8*64 = 512 fp32 = 2KB)
            for s in range(n_super):
                n0 = s * S
                ncnt = min(S, nrows - n0)
                ntiles = (ncnt + P - 1) // P
                xt0 = g_x.tile([P, S], F16, tag="xt0")
                xt1 = g_x.tile([P, S], F16, tag="xt1")
                nc.sync.dma_start(xt0[:, 0:ncnt], xT[0:P, n0 : n0 + ncnt])
                nc.sync.dma_start(xt1[:, 0:ncnt], xT[P : 2 * P, n0 : n0 + ncnt])
                hb = g_h.tile([P, (S // P) * U], F16, tag="hb")
                for tg in range(0, ntiles, TPG):
                    tn = min(TPG, ntiles - tg)
                    ps = g_ps.tile([P, TPG * U], F32)
                    for t in range(tg, tg + tn):
                        c0 = t * P
                        cw = min(P, ncnt - c0)
                        pslice = ps[0:cw, (t - tg) * U : (t - tg + 1) * U]
                        nc.tensor.matmul(
                            pslice, xt0[:, c0 : c0 + cw], wkt[:, 0:U],
                            start=True, stop=False,
                        )
                        nc.tensor.matmul(
                            pslice, xt1[:, c0 : c0 + cw], wkt[:, U : 2 * U],
                            start=False, stop=True,
                        )
                    # one batched fp32->fp16 copy per PSUM bank
                    nc.scalar.copy(
                        hb[:, tg * U : (tg + tn) * U], ps[:, 0 : tn * U]
                    )
                hb3 = hb[:].rearrange("p (t u) -> p t u", u=U)
                # write h rows [n0, n0+ncnt) (full 128-row tiles coalesced)
                lim0, lim1 = n0, n0 + ncnt
                ta = 0
                tb = (lim1 - n0) // P
                segs = []
                if tb > ta:
                    segs.append((n0, n0 + tb * P))
                if n0 + tb * P < lim1:
                    segs.append((n0 + tb * P, lim1))
                for r0, r1 in segs:
                    nt = (r1 - r0) // P
                    if nt >= 1 and (r0 - n0) % P == 0:
                        tt = (r0 - n0) // P
                        dr = h[r0:r1, :].rearrange("(o p) u -> p o u", p=P)
                        srcv = hb3[:, tt : tt + nt, :]
                    else:
                        tt = (r0 - n0) // P
                        p0 = r0 - (n0 + tt * P)
                        p1 = r1 - (n0 + tt * P)
                        dr = h[r0:r1, :].rearrange("(o p) u -> p o u", p=p1 - p0)
                        srcv = hb3[p0:p1, tt : tt + 1, :]
                    h_writes.append(nc.sync.dma_start(dr, srcv))

            # ---------------- phase 2: gather + select + median ----------------
            ctx1.close()
            g_net = ctx.enter_context(tc.tile_pool(name="g_net", bufs=net_bufs))
            g_srt = ctx.enter_context(tc.tile_pool(name="g_srt", bufs=2 * srt_sets))
            g_idx = ctx.enter_context(tc.tile_pool(name="g_idx", bufs=2))
            g_out = ctx.enter_context(tc.tile_pool(name="g_out", bufs=2))
            g_big = ctx.enter_context(tc.tile_pool(name="g_big", bufs=1))

            nc.gpsimd.load_library(library_config.mlp)
            med_all = g_big.tile([P, NCHUNK * B * U], F32, tag="medall")
            n_g = 0
            # per-call index count capped by the SWDGE ring
            KG = max(1, 1792 // C)
            kgroups = []
            k0 = 0
            while k0 < K:
                kgroups.append((k0, min(K, k0 + KG)))
                k0 += KG

            gat_tiles = {}

            def emit_tt(out, in0, in1, op):
                if pool_cols:
                    cs = out.shape[-1] - pool_cols
                    sl_d = (Ellipsis, slice(0, cs))
                    sl_p = (Ellipsis, slice(cs, out.shape[-1]))
                    nc.vector.tensor_tensor(
                        out=out[sl_d], in0=in0[sl_d], in1=in1[sl_d], op=op)
                    nc.gpsimd.tensor_tensor(
                        out=out[sl_p], in0=in0[sl_p], in1=in1[sl_p], op=op)
                else:
                    nc.vector.tensor_tensor(out=out, in0=in0, in1=in1, op=op)

            def emit_gather(c):
                nonlocal n_g
                if c >= NCHUNK or c in gat_tiles:
                    return
                ia = g_idx.tile([P, IDXCOLS], I16, tag="ia")
                nc.sync.dma_start(ia[:], idx[c])
                gat = g_net.tile([P, KB * 2 * U], F16, tag="gat")
                gat3 = gat[:].rearrange("p (j e) -> p j e", e=2 * U)
                for ka, kb in kgroups:
                    nidx = C * (kb - ka)
                    g = nc.gpsimd.dma_gather(
                        gat3[:, ka * B : kb * B, :],
                        h_pairs,
                        ia[:, ka * C // 16 : kb * C // 16],
                        nidx,
                        nidx,
                        2 * U,
                        single_packet=False,
                    )
                    if n_g == 0:
                        for w in h_writes:
                            add_dep_helper(
                                g.ins, w.ins,
                                reason="gather waits for h DRAM writes",
                            )
                    n_g += 1
                gat_tiles[c] = gat

            for c in range(lookahead):
                emit_gather(c)

            for c in range(NCHUNK):
                on_pool = pool_every and (c % pool_every == pool_every - 1)

                def VE(tag, _on_pool=on_pool):
                    return nc.gpsimd if (_on_pool or tag in pool_ops) else nc.vector

                emit_gather(c + lookahead)
                gat = gat_tiles.pop(c)
                gat3 = gat[:].rearrange("p (j e) -> p j e", e=2 * U)
                pa = g_idx.tile([P, KB], I16, tag="pa")
                nc.sync.dma_start(pa[:], par[c])
                # parity select in place: overwrite gat's E columns with O
                # where the neighbor is odd; stage 1 then reads the selected
                # (strided) columns directly.
                ra = g_srt.tile([P, K * BU], F16, tag="ra")
                rb = g_srt.tile([P, K * BU], F16, tag="rb")
                nc.vector.copy_predicated(
                    out=gat3[:, :, 0:U],
                    mask=pa[:].rearrange("p (j o) -> p j o", o=1).to_broadcast([P, KB, U]),
                    data=gat3[:, :, U : 2 * U],
                )

                # Batcher network; stage 1 reads gat (strided), writes ra;
                # stages 2+ ping-pong ra <-> rb
                gsel = gat[:].rearrange("p (hi r b e) -> p hi r b e", hi=16, r=2, e=2 * U)
                src, dst = rb, ra
                for si, sp in enumerate(SORT16_STAGES):
                    eng = VE(f"s{si}")
                    f = sp["f"]
                    ni = 16 // f
                    i_full = sp["i"] == (0, ni, 1)
                    d = sp["d"]
                    di, dr = d // f, d % f
                    r_vals = list(range(*sp["r"]))
                    if r_vals[-1] + dr >= f:
                        assert all(rv + dr >= f for rv in r_vals), sp
                        di, dr = di + 1, dr - f
                    r_sl = slice(*sp["r"])
                    hi_r = slice(sp["r"][0] + dr, sp["r"][1] + dr, sp["r"][2])
                    if i_full and di == 0:
                        if si == 0:
                            vs = None
                            vd = dst[:].rearrange(
                                "p (hi r b u) -> p hi r b u", hi=16, r=2, u=U
                            )
                            lo_s = gsel[:, :, r_sl, :, 0:U]
                            hi_s = gsel[:, :, hi_r, :, 0:U]
                            emit_tt(vd[:, :, r_sl, :, :], lo_s, hi_s,
                                    mybir.AluOpType.min)
                            emit_tt(vd[:, :, hi_r, :, :], lo_s, hi_s,
                                    mybir.AluOpType.max)
                            src, dst = dst, src
                            continue
                        vs = src[:].rearrange("p (hi r bu) -> p hi r bu", r=f, bu=BU)
                        vd = dst[:].rearrange("p (hi r bu) -> p hi r bu", r=f, bu=BU)
                        lo_s = vs[:, :, r_sl, :]
                        hi_s = vs[:, :, hi_r, :]
                        emit_tt(vd[:, :, r_sl, :], lo_s, hi_s,
                                mybir.AluOpType.min)
                        emit_tt(vd[:, :, hi_r, :], lo_s, hi_s,
                                mybir.AluOpType.max)
                    else:
                        i_sl = slice(*sp["i"])
                        hi_i = slice(sp["i"][0] + di, sp["i"][1] + di, sp["i"][2])
                        vs = src[:].rearrange(
                            "p (hh i r bu) -> p hh i r bu", hh=2, i=ni, r=f, bu=BU
                        )
                        vd = dst[:].rearrange(
                            "p (hh i r bu) -> p hh i r bu", hh=2, i=ni, r=f, bu=BU
                        )
                        lo_s = vs[:, :, i_sl, r_sl, :]
                        hi_s = vs[:, :, hi_i, hi_r, :]
                        emit_tt(vd[:, :, i_sl, r_sl, :], lo_s, hi_s,
                                mybir.AluOpType.min)
                        emit_tt(vd[:, :, hi_i, hi_r, :], lo_s, hi_s,
                                mybir.AluOpType.max)
                    vks = src[:].rearrange("p (hh kk bu) -> p hh kk bu", hh=2, kk=16)
                    vkd = dst[:].rearrange("p (hh kk bu) -> p hh kk bu", hh=2, kk=16)
                    for cpsl in sp["cp"]:
                        ks = slice(*cpsl)
                        nc.scalar.copy(vkd[:, :, ks, :], vks[:, :, ks, :])
                    src, dst = dst, src

                # anti-diagonal merge of the two sorted 16-plane halves
                vk = src[:].rearrange("p (k bu) -> p k bu", k=K)
                vo = dst[:].rearrange("p (k bu) -> p k bu", k=K)
                A = vk[:, 0:16, :]
                Brev = vk[:, 31:15:-1, :]
                emit_tt(vo[:, 0:16, :], A, Brev, mybir.AluOpType.max)
                emit_tt(vk[:, 0:16, :], A, Brev, mybir.AluOpType.min)
                # tree reductions over the 16 planes (packed fp16 TT ops):
                # low = max over min-planes (in vk), up = min over max-planes (vo)
                for buf, op in ((vk, mybir.AluOpType.max), (vo, mybir.AluOpType.min)):
                    w = 8
                    while w >= 1:
                        emit_tt(buf[:, 0:w, :], buf[:, 0:w, :],
                                buf[:, w : 2 * w, :], op)
                        w //= 2
                ms = med_all[:, c * BU : (c + 1) * BU]
                nc.vector.tensor_tensor(
                    out=ms, in0=vk[:, 0, :], in1=vo[:, 0, :], op=mybir.AluOpType.add
                )
                nc.scalar.mul(ms, ms, 0.5)
                nc.sync.dma_start(out[c], ms)

    nc.compile()
    return nc


def _prep_inputs(x, neighbors, kern, num_cores=NUM_CORES, C=CHUNK):
    import ml_dtypes

    nrows = x.shape[0]
    total = neighbors.shape[0]
    shard = (total + num_cores - 1) // num_cores
    NCHUNK = (shard + C - 1) // C
    shard_pad = NCHUNK * C
    B = C // P
    NIDX = C * K
    IDXCOLS = NIDX // 16
    KB = K * B

    xT = np.ascontiguousarray(x.T).astype(np.float16)
    wk = np.ascontiguousarray(kern).astype(np.float16)

    in_maps = []
    for core in range(num_cores):
        n0 = core * shard
        nbr = np.zeros((shard_pad, K), dtype=np.int64)
        real = min(shard, total - n0)
        nbr[:real] = neighbors[n0 : n0 + real]
        idxarr = np.empty((NCHUNK, P, IDXCOLS), dtype=np.int16)
        pararr = np.empty((NCHUNK, P, KB), dtype=np.int16)
        for c in range(NCHUNK):
            nb3 = nbr[c * C : (c + 1) * C].reshape(B, P, K)
            v = nb3.transpose(2, 0, 1).reshape(-1)  # i = ((k*B + b)*128 + p)
            pair = (v >> 1).astype(np.int16)
            # logical index i lives at [i%16, i//16]; replicated to all
            # eight 16-partition groups (Q7 core pairs read their own)
            idxarr[c] = np.tile(pair.reshape(IDXCOLS, 16).T, (P // 16, 1))
            pararr[c] = (v & 1).astype(np.int16).reshape(KB, P).T
        in_maps.append({"xT": xT, "wk": wk, "idx": idxarr, "par": pararr})
    meta = dict(shard=shard, shard_pad=shard_pad, NCHUNK=NCHUNK, C=C, total=total)
    return in_maps, meta


def _unshard_output(results, meta, num_cores=NUM_CORES):
    outs = []
    for core in range(num_cores):
        o = results[core]["out"]  # [NCHUNK, P, B*U]
        NCHUNK, _, BU_ = o.shape
        B = BU_ // U
        o = (
            o.reshape(NCHUNK, P, B, U)
            .transpose(0, 2, 1, 3)
            .reshape(meta["shard_pad"], U)
        )
        outs.append(o[: meta["shard"]])
    return np.concatenate(outs, axis=0)[: meta["total"]]


_CACHE = {}


def kernel(x, neighbors, kernel):
    """Full inputs in, full output out. Shards nodes across 8 NeuronCores."""
    x = np.asarray(x, dtype=np.float32)
    neighbors_np = np.asarray(neighbors)
    kern = np.asarray(kernel, dtype=np.float32)
    assert x.shape[1] == FEAT and kern.shape == (FEAT, U)
    assert neighbors_np.shape[1] == K

    in_maps, meta = _prep_inputs(x, neighbors_np, kern)
    key = (x.shape[0], meta["shard_pad"], meta["C"])
    if key not in _CACHE:
        _CACHE[key] = build_kernel(x.shape[0], meta["shard_pad"], meta["C"])
    nc = _CACHE[key]
    res = bass_utils.run_bass_kernel_spmd(
        nc, in_maps, core_ids=list(range(NUM_CORES))
    )
    return _unshard_output(res.results, meta)


# revision 6
# speedup vs baseline: 1.0181x; 1.0160x over previous
"""MedianConvolution (gnn message passing) — Trainium2 Bass kernel, 8 cores. v2

Computes: h = x @ kernel; msg = h[neighbors]; out = exact midpoint median
over the K=32 neighbor axis (ranks 15,16 of the sort).

v2 changes vs baseline:
  - h stored fp16 as row-PAIRS [nrows/2, 128]: one dma_gather per neighbor
    (idx = nbr>>1 fits int16; 256B blocks) instead of the lo/hi double
    gather -> gather DMA and Pool dispatch halved.
  - parity select on-chip: res = pair[0:64] ; copy_predicated overwrites
    with pair[64:128] where (nbr&1) using a host parity mask broadcast
    over units.
  - whole sort pipeline in fp16 (2x DVE throughput); x/kernel in bf16
    (halves phase-1 HBM traffic); PSUM->SBUF copies batched 8 col-tiles
    per ACT op.

Distribution: nodes sharded across 8 cores; every core computes the full
h = x @ kernel on-device (x replicated) and gathers its shard's rows.
"""
from contextlib import ExitStack

import numpy as np

import concourse.bass as bass
import concourse.tile as tile
from concourse import bacc, bass_utils, library_config, mybir
from concourse.tile_rust import add_dep_helper

F32 = mybir.dt.float32
F16 = mybir.dt.float16
BF16 = mybir.dt.bfloat16
I16 = mybir.dt.int16
U8 = mybir.dt.uint8
P = 128
U = 64  # units
K = 32  # neighbors
FEAT = 256
N_NODES = 50000
NUM_CORES = 8
CHUNK = 256  # shard nodes per chunk
NET_BUFS = 3
POOL_OPS = frozenset()

# Batcher odd-even mergesort(16) stages (verified via the 0-1 principle).
SORT16_STAGES = [
    dict(f=2, i=(0, 8, 1), r=(0, 1, 1), d=1, cp=[]),
    dict(f=4, i=(0, 4, 1), r=(0, 2, 1), d=2, cp=[]),
    dict(f=4, i=(0, 4, 1), r=(1, 2, 1), d=1, cp=[(0, 16, 4), (3, 16, 4)]),
    dict(f=8, i=(0, 2, 1), r=(0, 4, 1), d=4, cp=[]),
    dict(f=8, i=(0, 2, 1), r=(2, 4, 1), d=2,
         cp=[(0, 16, 8), (1, 16, 8), (6, 16, 8), (7, 16, 8)]),
    dict(f=8, i=(0, 2, 1), r=(1, 6, 2), d=1, cp=[(0, 16, 8), (7, 16, 8)]),
    dict(f=16, i=(0, 1, 1), r=(0, 8, 1), d=8, cp=[]),
    dict(f=16, i=(0, 1, 1), r=(4, 8, 1), d=4, cp=[(0, 4, 1), (12, 16, 1)]),
    dict(f=4, i=(0, 3, 1), r=(2, 4, 1), d=2, cp=[(0, 2, 1), (14, 16, 1)]),
    dict(f=2, i=(0, 7, 1), r=(1, 2, 1), d=1, cp=[(0, 16, 15)]),
]


def build_kernel(nrows, shard_nodes, C, num_cores=NUM_CORES, gemm_super=2048,
                 net_bufs=NET_BUFS, pool_ops=POOL_OPS, pool_every=0, lookahead=1,
                 pool_cols=0, srt_sets=2, XBUFS=3):
    assert nrows % 2 == 0
    HALF = nrows // 2
    NCHUNK = shard_nodes // C
    assert NCHUNK * C == shard_nodes
    B = C // P
    NIDX = C * K
    IDXCOLS = NIDX // 16
    BU = B * U          # sort plane width (fp16 elems)
    KB = K * B          # gather planes per chunk

    nc = bacc.Bacc(
        "TRN2",
        target_bir_lowering=False,
        debug=False,
        num_devices=num_cores,
    )

    xT = nc.dram_tensor("xT", [FEAT, nrows], F16, kind="ExternalInput").ap()
    wk = nc.dram_tensor("wk", [FEAT, U], F16, kind="ExternalInput").ap()
    idx = nc.dram_tensor("idx", [NCHUNK, P, IDXCOLS], I16, kind="ExternalInput").ap()
    par = nc.dram_tensor("par", [NCHUNK, P, KB], I16, kind="ExternalInput").ap()
    out = nc.dram_tensor("out", [NCHUNK, P, B * U], F32, kind="ExternalOutput").ap()
    # h rows fp16; gathered as row-pairs [HALF, 128]
    h = nc.dram_tensor("h", [nrows, U], F16, kind="Internal").ap()
    h_pairs = h.rearrange("(hh two) u -> hh (two u)", two=2)

    with tile.TileContext(nc) as tc:
        with ExitStack() as ctx:
            # ---------------- phase 1: GEMM ----------------
            ctx1 = ctx.enter_context(ExitStack())
            g_x = ctx1.enter_context(tc.tile_pool(name="g_x", bufs=XBUFS))
            g_w = ctx1.enter_context(tc.tile_pool(name="g_w", bufs=1))
            g_h = ctx1.enter_context(tc.tile_pool(name="g_h", bufs=2))
            g_ps = ctx1.enter_context(tc.tile_pool(name="g_ps", bufs=2, space="PSUM"))

            wkt = g_w.tile([P, 2 * U], F16)
            nc.sync.dma_start(wkt[:, 0:U], wk[0:P, :])
            nc.sync.dma_start(wkt[:, U : 2 * U], wk[P : 2 * P, :])

            h_writes = []
            S = gemm_super
            n_super = (nrows + S - 1) // S
            TPG = 8  # col-tiles per PSUM bank (8*64 = 512 fp32 = 2KB)
            for s in range(n_super):
                n0 = s * S
                ncnt = min(S, nrows - n0)
                ntiles = (ncnt + P - 1) // P
                xt0 = g_x.tile([P, S], F16, tag="xt0")
                xt1 = g_x.tile([P, S], F16, tag="xt1")
                nc.sync.dma_start(xt0[:, 0:ncnt], xT[0:P, n0 : n0 + ncnt])
                nc.sync.dma_start(xt1[:, 0:ncnt], xT[P : 2 * P, n0 : n0 + ncnt])
                hb = g_h.tile([P, (S // P) * U], F16, tag="hb")
                for tg in range(0, ntiles, TPG):
                    tn = min(TPG, ntiles - tg)
                    ps = g_ps.tile([P, TPG * U], F32)
                    for t in range(tg, tg + tn):
                        c0 = t * P
                        cw = min(P, ncnt - c0)
                        pslice = ps[0:cw, (t - tg) * U : (t - tg + 1) * U]
                        nc.tensor.matmul(
                            pslice, xt0[:, c0 : c0 + cw], wkt[:, 0:U],
                            start=True, stop=False,
                        )
                        nc.tensor.matmul(
                            pslice, xt1[:, c0 : c0 + cw], wkt[:, U : 2 * U],
                            start=False, stop=True,
                        )
                    # one batched fp32->fp16 copy per PSUM bank
                    nc.scalar.copy(
                        hb[:, tg * U : (tg + tn) * U], ps[:, 0 : tn * U]
                    )
                hb3 = hb[:].rearrange("p (t u) -> p t u", u=U)
                # write h rows [n0, n0+ncnt) (full 128-row tiles coalesced)
                lim0, lim1 = n0, n0 + ncnt
                ta = 0
                tb = (lim1 - n0) // P
                segs = []
                if tb > ta:
                    segs.append((n0, n0 + tb * P))
                if n0 + tb * P < lim1:
                    segs.append((n0 + tb * P, lim1))
                for r0, r1 in segs:
                    nt = (r1 - r0) // P
                    if nt >= 1 and (r0 - n0) % P == 0:
                        tt = (r0 - n0) // P
                        dr = h[r0:r1, :].rearrange("(o p) u -> p o u", p=P)
                        srcv = hb3[:, tt : tt + nt, :]
                    else:
                        tt = (r0 - n0) // P
                        p0 = r0 - (n0 + tt * P)
                        p1 = r1 - (n0 + tt * P)
                        dr = h[r0:r1, :].rearrange("(o p) u -> p o u", p=p1 - p0)
                        srcv = hb3[p0:p1, tt : tt + 1, :]
                    h_writes.append(nc.sync.dma_start(dr, srcv))

            # ---------------- phase 2: gather + select + median ----------------
            ctx1.close()
            g_net = ctx.enter_context(tc.tile_pool(name="g_net", bufs=net_bufs))
            g_srt = ctx.enter_context(tc.tile_pool(name="g_srt", bufs=2 * srt_sets))
            g_idx = ctx.enter_context(tc.tile_pool(name="g_idx", bufs=2))
            g_out = ctx.enter_context(tc.tile_pool(name="g_out", bufs=2))
            g_big = ctx.enter_context(tc.tile_pool(name="g_big", bufs=1))

            nc.gpsimd.load_library(library_config.mlp)
            med_all = g_big.tile([P, NCHUNK * B * U], F32, tag="medall")
            n_g = 0
            # per-call index count capped by the SWDGE ring
            KG = max(1, 1792 // C)
            kgroups = []
            k0 = 0
            while k0 < K:
                kgroups.append((k0, min(K, k0 + KG)))
                k0 += KG

            gat_tiles = {}

            def emit_tt(out, in0, in1, op):
                if pool_cols:
                    cs = out.shape[-1] - pool_cols
                    sl_d = (Ellipsis, slice(0, cs))
                    sl_p = (Ellipsis, slice(cs, out.shape[-1]))
                    nc.vector.tensor_tensor(
                        out=out[sl_d], in0=in0[sl_d], in1=in1[sl_d], op=op)
                    nc.gpsimd.tensor_tensor(
                        out=out[sl_p], in0=in0[sl_p], in1=in1[sl_p], op=op)
                else:
                    nc.vector.tensor_tensor(out=out, in0=in0, in1=in1, op=op)

            def emit_gather(c):
                nonlocal n_g
                if c >= NCHUNK or c in gat_tiles:
                    return
                ia = g_idx.tile([P, IDXCOLS], I16, tag="ia")
                nc.sync.dma_start(ia[:], idx[c])
                gat = g_net.tile([P, KB * 2 * U], F16, tag="gat")
                gat3 = gat[:].rearrange("p (j e) -> p j e", e=2 * U)
                for ka, kb in kgroups:
                    nidx = C * (kb - ka)
                    g = nc.gpsimd.dma_gather(
                        gat3[:, ka * B : kb * B, :],
                        h_pairs,
                        ia[:, ka * C // 16 : kb * C // 16],
                        nidx,
                        nidx,
                        2 * U,
                        single_packet=False,
                    )
                    if n_g == 0:
                        for w in h_writes:
                            add_dep_helper(
                                g.ins, w.ins,
                                reason="gather waits for h DRAM writes",
                            )
                    n_g += 1
                gat_tiles[c] = gat

            for c in range(lookahead):
                emit_gather(c)

            for c in range(NCHUNK):
                on_pool = pool_every and (c % pool_every == pool_every - 1)

                def VE(tag, _on_pool=on_pool):
                    return nc.gpsimd if (_on_pool or tag in pool_ops) else nc.vector

                emit_gather(c + lookahead)
                gat = gat_tiles.pop(c)
                gat3 = gat[:].rearrange("p (j e) -> p j e", e=2 * U)
                pa = g_idx.tile([P, KB], I16, tag="pa")
                nc.sync.dma_start(pa[:], par[c])
                # parity select in place: overwrite gat's E columns with O
                # where the neighbor is odd; stage 1 then reads the selected
                # (strided) columns directly.
                ra = g_srt.tile([P, K * BU], F16, tag="ra")
                rb = g_srt.tile([P, K * BU], F16, tag="rb")
                nc.vector.copy_predicated(
                    out=gat3[:, :, 0:U],
                    mask=pa[:].rearrange("p (j o) -> p j o", o=1).to_broadcast([P, KB, U]),
                    data=gat3[:, :, U : 2 * U],
                )

                # Batcher network; stage 1 reads gat (strided), writes ra;
                # stages 2+ ping-pong ra <-> rb
                gsel = gat[:].rearrange("p (hi r b e) -> p hi r b e", hi=16, r=2, e=2 * U)
                src, dst = rb, ra
                for si, sp in enumerate(SORT16_STAGES):
                    eng = VE(f"s{si}")
                    f = sp["f"]
                    ni = 16 // f
                    i_full = sp["i"] == (0, ni, 1)
                    d = sp["d"]
                    di, dr = d // f, d % f
                    r_vals = list(range(*sp["r"]))
                    if r_vals[-1] + dr >= f:
                        assert all(rv + dr >= f for rv in r_vals), sp
                        di, dr = di + 1, dr - f
                    r_sl = slice(*sp["r"])
                    hi_r = slice(sp["r"][0] + dr, sp["r"][1] + dr, sp["r"][2])
                    if i_full and di == 0:
                        if si == 0:
                            vs = None
                            vd = dst[:].rearrange(
                                "p (hi r b u) -> p hi r b u", hi=16, r=2, u=U
                            )
                            lo_s = gsel[:, :, r_sl, :, 0:U]
                            hi_s = gsel[:, :, hi_r, :, 0:U]
                            emit_tt(vd[:, :, r_sl, :, :], lo_s, hi_s,
                                    mybir.AluOpType.min)
                            emit_tt(vd[:, :, hi_r, :, :], lo_s, hi_s,
                                    mybir.AluOpType.max)
                            src, dst = dst, src
                            continue
                        vs = src[:].rearrange("p (hi r bu) -> p hi r bu", r=f, bu=BU)
                        vd = dst[:].rearrange("p (hi r bu) -> p hi r bu", r=f, bu=BU)
                        lo_s = vs[:, :, r_sl, :]
                        hi_s = vs[:, :, hi_r, :]
                        emit_tt(vd[:, :, r_sl, :], lo_s, hi_s,
                                mybir.AluOpType.min)
                        emit_tt(vd[:, :, hi_r, :], lo_s, hi_s,
                                mybir.AluOpType.max)
                    else:
                        i_sl = slice(*sp["i"])
                        hi_i = slice(sp["i"][0] + di, sp["i"][1] + di, sp["i"][2])
                        vs = src[:].rearrange(
                            "p (hh i r bu) -> p hh i r bu", hh=2, i=ni, r=f, bu=BU
                        )
                        vd = dst[:].rearrange(
                            "p (hh i r bu) -> p hh i r bu", hh=2, i=ni, r=f, bu=BU
                        )
                        lo_s = vs[:, :, i_sl, r_sl, :]
                        hi_s = vs[:, :, hi_i, hi_r, :]
                        emit_tt(vd[:, :, i_sl, r_sl, :], lo_s, hi_s,
                                mybir.AluOpType.min)
                        emit_tt(vd[:, :, hi_i, hi_r, :], lo_s, hi_s,
                                mybir.AluOpType.max)
                    vks = src[:].rearrange("p (hh kk bu) -> p hh kk bu", hh=2, kk=16)
                    vkd = dst[:].rearrange("p (hh kk bu) -> p hh kk bu", hh=2, kk=16)
                    for cpsl in sp["cp"]:
                        ks = slice(*cpsl)
                        nc.scalar.copy(vkd[:, :, ks, :], vks[:, :, ks, :])
                    src, dst = dst, src

                # anti-diagonal merge of the two sorted 16-plane halves
                vk = src[:].rearrange("p (k bu) -> p k bu", k=K)
                vo = dst[:].rearrange("p (k bu) -> p k bu", k=K)
                A = vk[:, 0:16, :]
                Brev = vk[:, 31:15:-1, :]
                emit_tt(vo[:, 0:16, :], A, Brev, mybir.AluOpType.max)
                emit_tt(vk[:, 0:16, :], A, Brev, mybir.AluOpType.min)
                # tree reductions over the 16 planes (packed fp16 TT ops):
                # low = max over min-planes (in vk), up = min over max-planes (vo)
                for buf, op in ((vk, mybir.AluOpType.max), (vo, mybir.AluOpType.min)):
                    w = 8
                    while w >= 1:
                        emit_tt(buf[:, 0:w, :], buf[:, 0:w, :],
                                buf[:, w : 2 * w, :], op)
                        w //= 2
                ms = med_all[:, c * BU : (c + 1) * BU]
                nc.vector.tensor_tensor(
                    out=ms, in0=vk[:, 0, :], in1=vo[:, 0, :], op=mybir.AluOpType.add
                )
                nc.scalar.mul(ms, ms, 0.5)
                nc.sync.dma_start(out[c], ms)

    nc.compile()
    return nc


def _prep_inputs(x, neighbors, kern, num_cores=NUM_CORES, C=CHUNK):
    nrows = x.shape[0]
    total = neighbors.shape[0]
    shard = (total + num_cores - 1) // num_cores
    NCHUNK = (shard + C - 1) // C
    shard_pad = NCHUNK * C
    B = C // P
    NIDX = C * K
    IDXCOLS = NIDX // 16
    KB = K * B

    xT = np.ascontiguousarray(x.T).astype(np.float16)
    wk = np.ascontiguousarray(kern).astype(np.float16)

    in_maps = []
    for core in range(num_cores):
        n0 = core * shard
        nbr = np.zeros((shard_pad, K), dtype=np.int64)
        real = min(shard, total - n0)
        nbr[:real] = neighbors[n0 : n0 + real]
        idxarr = np.empty((NCHUNK, P, IDXCOLS), dtype=np.int16)
        pararr = np.empty((NCHUNK, P, KB), dtype=np.int16)
        for c in range(NCHUNK):
            nb3 = nbr[c * C : (c + 1) * C].reshape(B, P, K)
            v = nb3.transpose(2, 0, 1).reshape(-1)  # i = ((k*B + b)*128 + p)
            pair = (v >> 1).astype(np.int16)
            # logical index i lives at [i%16, i//16]; replicated to all
            # eight 16-partition groups (Q7 core pairs read their own)
            idxarr[c] = np.tile(pair.reshape(IDXCOLS, 16).T, (P // 16, 1))
            pararr[c] = (v & 1).astype(np.int16).reshape(KB, P).T
        in_maps.append({"xT": xT, "wk": wk, "idx": idxarr, "par": pararr})
    meta = dict(shard=shard, shard_pad=shard_pad, NCHUNK=NCHUNK, C=C, total=total)
    return in_maps, meta


def _unshard_output(results, meta, num_cores=NUM_CORES):
    outs = []
    for core in range(num_cores):
        o = results[core]["out"]  # [NCHUNK, P, B*U]
        NCHUNK, _, BU_ = o.shape
        B = BU_ // U
        o = (
            o.reshape(NCHUNK, P, B, U)
            .transpose(0, 2, 1, 3)
            .reshape(meta["shard_pad"], U)
        )
        outs.append(o[: meta["shard"]])
    return np.concatenate(outs, axis=0)[: meta["total"]]


_CACHE = {}


def kernel(x, neighbors, kernel):
    """Full inputs in, full output out. Shards nodes across 8 NeuronCores."""
    x = np.asarray(x, dtype=np.float32)
    neighbors_np = np.asarray(neighbors)
    kern = np.asarray(kernel, dtype=np.float32)
    assert x.shape[1] == FEAT and kern.shape == (FEAT, U)
    assert neighbors_np.shape[1] == K

    in_maps, meta = _prep_inputs(x, neighbors_np, kern)
    key = (x.shape[0], meta["shard_pad"], meta["C"])
    if key not in _CACHE:
        _CACHE[key] = build_kernel(x.shape[0], meta["shard_pad"], meta["C"])
    nc = _CACHE[key]
    res = bass_utils.run_bass_kernel_spmd(
        nc, in_maps, core_ids=list(range(NUM_CORES))
    )
    return _unshard_output(res.results, meta)


# revision 7
# speedup vs baseline: 1.0292x; 1.0108x over previous
"""MedianConvolution (gnn message passing) — Trainium2 Bass kernel, 8 cores. v2

Computes: h = x @ kernel; msg = h[neighbors]; out = exact midpoint median
over the K=32 neighbor axis (ranks 15,16 of the sort).

v2 changes vs baseline:
  - h stored fp16 as row-PAIRS [nrows/2, 128]: one dma_gather per neighbor
    (idx = nbr>>1 fits int16; 256B blocks) instead of the lo/hi double
    gather -> gather DMA and Pool dispatch halved.
  - parity select on-chip: res = pair[0:64] ; copy_predicated overwrites
    with pair[64:128] where (nbr&1) using a host parity mask broadcast
    over units.
  - whole sort pipeline in fp16 (2x DVE throughput); x/kernel in bf16
    (halves phase-1 HBM traffic); PSUM->SBUF copies batched 8 col-tiles
    per ACT op.

Distribution: nodes sharded across 8 cores; every core computes the full
h = x @ kernel on-device (x replicated) and gathers its shard's rows.
"""
from contextlib import ExitStack

import numpy as np

import concourse.bass as bass
import concourse.tile as tile
from concourse import bacc, bass_utils, library_config, mybir
from concourse.tile_rust import add_dep_helper

F32 = mybir.dt.float32
F16 = mybir.dt.float16
BF16 = mybir.dt.bfloat16
I16 = mybir.dt.int16
U8 = mybir.dt.uint8
P = 128
U = 64  # units
K = 32  # neighbors
FEAT = 256
N_NODES = 50000
NUM_CORES = 8
CHUNK = 256  # shard nodes per chunk
NET_BUFS = 3
POOL_OPS = frozenset()

def chunk_plan(shard, C=CHUNK):
    """[(start_node, C_c, B_c)] covering >= shard nodes, 128-aligned; full
    C-sized chunks plus one minimal tail chunk (cuts pad-node sort waste)."""
    plan = []
    n = 0
    while shard - n >= C:
        plan.append((n, C, C // P))
        n += C
    if n < shard:
        rem = shard - n
        Cc = ((rem + P - 1) // P) * P
        plan.append((n, Cc, Cc // P))
    return plan


# Batcher odd-even mergesort(16) stages (verified via the 0-1 principle).
SORT16_STAGES = [
    dict(f=2, i=(0, 8, 1), r=(0, 1, 1), d=1, cp=[]),
    dict(f=4, i=(0, 4, 1), r=(0, 2, 1), d=2, cp=[]),
    dict(f=4, i=(0, 4, 1), r=(1, 2, 1), d=1, cp=[(0, 16, 4), (3, 16, 4)]),
    dict(f=8, i=(0, 2, 1), r=(0, 4, 1), d=4, cp=[]),
    dict(f=8, i=(0, 2, 1), r=(2, 4, 1), d=2,
         cp=[(0, 16, 8), (1, 16, 8), (6, 16, 8), (7, 16, 8)]),
    dict(f=8, i=(0, 2, 1), r=(1, 6, 2), d=1, cp=[(0, 16, 8), (7, 16, 8)]),
    dict(f=16, i=(0, 1, 1), r=(0, 8, 1), d=8, cp=[]),
    dict(f=16, i=(0, 1, 1), r=(4, 8, 1), d=4, cp=[(0, 4, 1), (12, 16, 1)]),
    dict(f=4, i=(0, 3, 1), r=(2, 4, 1), d=2, cp=[(0, 2, 1), (14, 16, 1)]),
    dict(f=2, i=(0, 7, 1), r=(1, 2, 1), d=1, cp=[(0, 16, 15)]),
]


def build_kernel(nrows, shard_nodes, C, num_cores=NUM_CORES, gemm_super=2048,
                 net_bufs=NET_BUFS, pool_ops=POOL_OPS, pool_every=0, lookahead=1,
                 pool_cols=0, srt_sets=2, XBUFS=3):
    assert nrows % 2 == 0
    HALF = nrows // 2
    plan = chunk_plan(shard_nodes, C)
    NCHUNK = len(plan)
    B = C // P
    NIDX = C * K
    IDXCOLS = NIDX // 16
    BU = B * U          # sort plane width (fp16 elems), full-size chunks
    KB = K * B          # gather planes per chunk, full-size chunks

    nc = bacc.Bacc(
        "TRN2",
        target_bir_lowering=False,
        debug=False,
        num_devices=num_cores,
    )

    xT = nc.dram_tensor("xT", [FEAT, nrows], F16, kind="ExternalInput").ap()
    wk = nc.dram_tensor("wk", [FEAT, U], F16, kind="ExternalInput").ap()
    idx = nc.dram_tensor("idx", [NCHUNK, P, IDXCOLS], I16, kind="ExternalInput").ap()
    par = nc.dram_tensor("par", [NCHUNK, P, KB], I16, kind="ExternalInput").ap()
    out = nc.dram_tensor("out", [NCHUNK, P, B * U], F32, kind="ExternalOutput").ap()
    # h rows fp16; gathered as row-pairs [HALF, 128]
    h = nc.dram_tensor("h", [nrows, U], F16, kind="Internal").ap()
    h_pairs = h.rearrange("(hh two) u -> hh (two u)", two=2)

    with tile.TileContext(nc) as tc:
        with ExitStack() as ctx:
            # ---------------- phase 1: GEMM ----------------
            ctx1 = ctx.enter_context(ExitStack())
            g_x = ctx1.enter_context(tc.tile_pool(name="g_x", bufs=XBUFS))
            g_w = ctx1.enter_context(tc.tile_pool(name="g_w", bufs=1))
            g_h = ctx1.enter_context(tc.tile_pool(name="g_h", bufs=2))
            g_ps = ctx1.enter_context(tc.tile_pool(name="g_ps", bufs=2, space="PSUM"))

            wkt = g_w.tile([P, 2 * U], F16)
            nc.sync.dma_start(wkt[:, 0:U], wk[0:P, :])
            nc.sync.dma_start(wkt[:, U : 2 * U], wk[P : 2 * P, :])

            h_writes = []
            S = gemm_super
            n_super = (nrows + S - 1) // S
            TPG = 8  # col-tiles per PSUM bank (8*64 = 512 fp32 = 2KB)
            for s in range(n_super):
                n0 = s * S
                ncnt = min(S, nrows - n0)
                ntiles = (ncnt + P - 1) // P
                xt0 = g_x.tile([P, S], F16, tag="xt0")
                xt1 = g_x.tile([P, S], F16, tag="xt1")
                nc.sync.dma_start(xt0[:, 0:ncnt], xT[0:P, n0 : n0 + ncnt])
                nc.sync.dma_start(xt1[:, 0:ncnt], xT[P : 2 * P, n0 : n0 + ncnt])
                hb = g_h.tile([P, (S // P) * U], F16, tag="hb")
                for tg in range(0, ntiles, TPG):
                    tn = min(TPG, ntiles - tg)
                    ps = g_ps.tile([P, TPG * U], F32)
                    for t in range(tg, tg + tn):
                        c0 = t * P
                        cw = min(P, ncnt - c0)
                        pslice = ps[0:cw, (t - tg) * U : (t - tg + 1) * U]
                        nc.tensor.matmul(
                            pslice, xt0[:, c0 : c0 + cw], wkt[:, 0:U],
                            start=True, stop=False,
                        )
                        nc.tensor.matmul(
                            pslice, xt1[:, c0 : c0 + cw], wkt[:, U : 2 * U],
                            start=False, stop=True,
                        )
                    # one batched fp32->fp16 copy per PSUM bank
                    nc.scalar.copy(
                        hb[:, tg * U : (tg + tn) * U], ps[:, 0 : tn * U]
                    )
                hb3 = hb[:].rearrange("p (t u) -> p t u", u=U)
                # write h rows [n0, n0+ncnt) (full 128-row tiles coalesced)
                lim0, lim1 = n0, n0 + ncnt
                ta = 0
                tb = (lim1 - n0) // P
                segs = []
                if tb > ta:
                    segs.append((n0, n0 + tb * P))
                if n0 + tb * P < lim1:
                    segs.append((n0 + tb * P, lim1))
                for r0, r1 in segs:
                    nt = (r1 - r0) // P
                    if nt >= 1 and (r0 - n0) % P == 0:
                        tt = (r0 - n0) // P
                        dr = h[r0:r1, :].rearrange("(o p) u -> p o u", p=P)
                        srcv = hb3[:, tt : tt + nt, :]
                    else:
                        tt = (r0 - n0) // P
                        p0 = r0 - (n0 + tt * P)
                        p1 = r1 - (n0 + tt * P)
                        dr = h[r0:r1, :].rearrange("(o p) u -> p o u", p=p1 - p0)
                        srcv = hb3[p0:p1, tt : tt + 1, :]
                    h_writes.append(nc.sync.dma_start(dr, srcv))

            # ---------------- phase 2: gather + select + median ----------------
            ctx1.close()
            g_net = ctx.enter_context(tc.tile_pool(name="g_net", bufs=net_bufs))
            g_srt = ctx.enter_context(tc.tile_pool(name="g_srt", bufs=2 * srt_sets))
            g_idx = ctx.enter_context(tc.tile_pool(name="g_idx", bufs=2))
            g_out = ctx.enter_context(tc.tile_pool(name="g_out", bufs=2))
            g_big = ctx.enter_context(tc.tile_pool(name="g_big", bufs=1))

            nc.gpsimd.load_library(library_config.mlp)
            med_all = g_big.tile([P, NCHUNK * B * U], F32, tag="medall")
            n_g = 0
            # per-call index count capped by the SWDGE ring
            KG = max(1, 1792 // C)
            kgroups = []
            k0 = 0
            while k0 < K:
                kgroups.append((k0, min(K, k0 + KG)))
                k0 += KG

            gat_tiles = {}

            def emit_tt(out, in0, in1, op):
                if pool_cols:
                    cs = out.shape[-1] - pool_cols
                    sl_d = (Ellipsis, slice(0, cs))
                    sl_p = (Ellipsis, slice(cs, out.shape[-1]))
                    nc.vector.tensor_tensor(
                        out=out[sl_d], in0=in0[sl_d], in1=in1[sl_d], op=op)
                    nc.gpsimd.tensor_tensor(
                        out=out[sl_p], in0=in0[sl_p], in1=in1[sl_p], op=op)
                else:
                    nc.vector.tensor_tensor(out=out, in0=in0, in1=in1, op=op)

            def emit_gather(c):
                nonlocal n_g
                if c >= NCHUNK or c in gat_tiles:
                    return
                _, Cc, Bc = plan[c]
                KBc = K * Bc
                IDXCOLSc = Cc * K // 16
                KGc = max(1, 1792 // Cc)
                kgroups_c = [(k0, min(K, k0 + KGc)) for k0 in range(0, K, KGc)]
                ia = g_idx.tile([P, IDXCOLS], I16, tag="ia")
                nc.sync.dma_start(ia[:, 0:IDXCOLSc], idx[c, :, 0:IDXCOLSc])
                gat = g_net.tile([P, KB * 2 * U], F16, tag="gat")
                gat3 = gat[:, 0 : KBc * 2 * U].rearrange("p (j e) -> p j e", e=2 * U)
                for ka, kb in kgroups_c:
                    nidx = Cc * (kb - ka)
                    g = nc.gpsimd.dma_gather(
                        gat3[:, ka * Bc : kb * Bc, :],
                        h_pairs,
                        ia[:, ka * Cc // 16 : kb * Cc // 16],
                        nidx,
                        nidx,
                        2 * U,
                        single_packet=False,
                    )
                    if n_g == 0:
                        for w in h_writes:
                            add_dep_helper(
                                g.ins, w.ins,
                                reason="gather waits for h DRAM writes",
                            )
                    n_g += 1
                gat_tiles[c] = gat

            for c in range(lookahead):
                emit_gather(c)

            for c in range(NCHUNK):
                on_pool = pool_every and (c % pool_every == pool_every - 1)

                def VE(tag, _on_pool=on_pool):
                    return nc.gpsimd if (_on_pool or tag in pool_ops) else nc.vector

                _, Cc, Bc = plan[c]
                BUc = Bc * U
                KBc = K * Bc
                emit_gather(c + lookahead)
                gat = gat_tiles.pop(c)
                gat3 = gat[:, 0 : KBc * 2 * U].rearrange("p (j e) -> p j e", e=2 * U)
                pa = g_idx.tile([P, KB], I16, tag="pa")
                nc.sync.dma_start(pa[:, 0:KBc], par[c, :, 0:KBc])
                # parity select in place: overwrite gat's E columns with O
                # where the neighbor is odd; stage 1 then reads the selected
                # (strided) columns directly.
                ra = g_srt.tile([P, K * BU], F16, tag="ra")
                rb = g_srt.tile([P, K * BU], F16, tag="rb")
                nc.vector.copy_predicated(
                    out=gat3[:, :, 0:U],
                    mask=pa[:, 0:KBc].rearrange("p (j o) -> p j o", o=1)
                    .to_broadcast([P, KBc, U]),
                    data=gat3[:, :, U : 2 * U],
                )

                # Batcher network; stage 1 reads gat (strided), writes ra;
                # stages 2+ ping-pong ra <-> rb
                gsel = gat[:, 0 : KBc * 2 * U].rearrange(
                    "p (hi r b e) -> p hi r b e", hi=16, r=2, b=Bc, e=2 * U
                )
                src, dst = rb, ra
                for si, sp in enumerate(SORT16_STAGES):
                    eng = VE(f"s{si}")
                    f = sp["f"]
                    ni = 16 // f
                    i_full = sp["i"] == (0, ni, 1)
                    d = sp["d"]
                    di, dr = d // f, d % f
                    r_vals = list(range(*sp["r"]))
                    if r_vals[-1] + dr >= f:
                        assert all(rv + dr >= f for rv in r_vals), sp
                        di, dr = di + 1, dr - f
                    r_sl = slice(*sp["r"])
                    hi_r = slice(sp["r"][0] + dr, sp["r"][1] + dr, sp["r"][2])
                    if i_full and di == 0:
                        if si == 0:
                            vs = None
                            vd = dst[:, 0 : K * BUc].rearrange(
                                "p (hi r b u) -> p hi r b u", hi=16, r=2, b=Bc, u=U
                            )
                            lo_s = gsel[:, :, r_sl, :, 0:U]
                            hi_s = gsel[:, :, hi_r, :, 0:U]
                            emit_tt(vd[:, :, r_sl, :, :], lo_s, hi_s,
                                    mybir.AluOpType.min)
                            emit_tt(vd[:, :, hi_r, :, :], lo_s, hi_s,
                                    mybir.AluOpType.max)
                            src, dst = dst, src
                            continue
                        vs = src[:, 0 : K * BUc].rearrange(
                            "p (hi r bu) -> p hi r bu", r=f, bu=BUc)
                        vd = dst[:, 0 : K * BUc].rearrange(
                            "p (hi r bu) -> p hi r bu", r=f, bu=BUc)
                        lo_s = vs[:, :, r_sl, :]
                        hi_s = vs[:, :, hi_r, :]
                        emit_tt(vd[:, :, r_sl, :], lo_s, hi_s,
                                mybir.AluOpType.min)
                        emit_tt(vd[:, :, hi_r, :], lo_s, hi_s,
                                mybir.AluOpType.max)
                    else:
                        i_sl = slice(*sp["i"])
                        hi_i = slice(sp["i"][0] + di, sp["i"][1] + di, sp["i"][2])
                        vs = src[:, 0 : K * BUc].rearrange(
                            "p (hh i r bu) -> p hh i r bu", hh=2, i=ni, r=f, bu=BUc
                        )
                        vd = dst[:, 0 : K * BUc].rearrange(
                            "p (hh i r bu) -> p hh i r bu", hh=2, i=ni, r=f, bu=BUc
                        )
                        lo_s = vs[:, :, i_sl, r_sl, :]
                        hi_s = vs[:, :, hi_i, hi_r, :]
                        emit_tt(vd[:, :, i_sl, r_sl, :], lo_s, hi_s,
                                mybir.AluOpType.min)
                        emit_tt(vd[:, :, hi_i, hi_r, :], lo_s, hi_s,
                                mybir.AluOpType.max)
                    vks = src[:, 0 : K * BUc].rearrange(
                        "p (hh kk bu) -> p hh kk bu", hh=2, kk=16)
                    vkd = dst[:, 0 : K * BUc].rearrange(
                        "p (hh kk bu) -> p hh kk bu", hh=2, kk=16)
                    for cpsl in sp["cp"]:
                        ks = slice(*cpsl)
                        nc.scalar.copy(vkd[:, :, ks, :], vks[:, :, ks, :])
                    src, dst = dst, src

                # anti-diagonal merge of the two sorted 16-plane halves
                vk = src[:, 0 : K * BUc].rearrange("p (k bu) -> p k bu", k=K)
                vo = dst[:, 0 : K * BUc].rearrange("p (k bu) -> p k bu", k=K)
                A = vk[:, 0:16, :]
                Brev = vk[:, 31:15:-1, :]
                emit_tt(vo[:, 0:16, :], A, Brev, mybir.AluOpType.max)
                emit_tt(vk[:, 0:16, :], A, Brev, mybir.AluOpType.min)
                # tree reductions over the 16 planes (packed fp16 TT ops):
                # low = max over min-planes (in vk), up = min over max-planes (vo)
                for buf, op in ((vk, mybir.AluOpType.max), (vo, mybir.AluOpType.min)):
                    w = 8
                    while w >= 1:
                        emit_tt(buf[:, 0:w, :], buf[:, 0:w, :],
                                buf[:, w : 2 * w, :], op)
                        w //= 2
                ms = med_all[:, c * BU : c * BU + BUc]
                nc.vector.tensor_tensor(
                    out=ms, in0=vk[:, 0, :], in1=vo[:, 0, :], op=mybir.AluOpType.add
                )
                nc.scalar.mul(ms, ms, 0.5)
                nc.sync.dma_start(out[c, :, 0:BUc], ms)

    nc.compile()
    return nc


def _prep_inputs(x, neighbors, kern, num_cores=NUM_CORES, C=CHUNK):
    nrows = x.shape[0]
    total = neighbors.shape[0]
    shard = (total + num_cores - 1) // num_cores
    plan = chunk_plan(shard, C)
    NCHUNK = len(plan)
    shard_pad = plan[-1][0] + plan[-1][1]
    B = C // P
    NIDX = C * K
    IDXCOLS = NIDX // 16
    KB = K * B

    xT = np.ascontiguousarray(x.T).astype(np.float16)
    wk = np.ascontiguousarray(kern).astype(np.float16)

    in_maps = []
    for core in range(num_cores):
        n0 = core * shard
        nbr = np.zeros((shard_pad, K), dtype=np.int64)
        real = min(shard, total - n0)
        nbr[:real] = neighbors[n0 : n0 + real]
        idxarr = np.zeros((NCHUNK, P, IDXCOLS), dtype=np.int16)
        pararr = np.zeros((NCHUNK, P, KB), dtype=np.int16)
        for c, (n0c, Cc, Bc) in enumerate(plan):
            IDXCOLSc = Cc * K // 16
            KBc = K * Bc
            nb3 = nbr[n0c : n0c + Cc].reshape(Bc, P, K)
            v = nb3.transpose(2, 0, 1).reshape(-1)  # i = ((k*Bc + b)*128 + p)
            pair = (v >> 1).astype(np.int16)
            # logical index i lives at [i%16, i//16]; replicated to all
            # eight 16-partition groups (Q7 core pairs read their own)
            idxarr[c, :, 0:IDXCOLSc] = np.tile(
                pair.reshape(IDXCOLSc, 16).T, (P // 16, 1)
            )
            pararr[c, :, 0:KBc] = (v & 1).astype(np.int16).reshape(KBc, P).T
        in_maps.append({"xT": xT, "wk": wk, "idx": idxarr, "par": pararr})
    meta = dict(shard=shard, shard_pad=shard_pad, NCHUNK=NCHUNK, C=C, total=total,
                plan=plan)
    return in_maps, meta


def _unshard_output(results, meta, num_cores=NUM_CORES):
    outs = []
    for core in range(num_cores):
        o = results[core]["out"]  # [NCHUNK, P, B*U]
        full = np.empty((meta["shard_pad"], U), dtype=o.dtype)
        for c, (n0c, Cc, Bc) in enumerate(meta["plan"]):
            oc = o[c, :, 0 : Bc * U].reshape(P, Bc, U)
            full[n0c : n0c + Cc] = oc.transpose(1, 0, 2).reshape(Cc, U)
        outs.append(full[: meta["shard"]])
    return np.concatenate(outs, axis=0)[: meta["total"]]


_CACHE = {}


def kernel(x, neighbors, kernel):
    """Full inputs in, full output out. Shards nodes across 8 NeuronCores."""
    x = np.asarray(x, dtype=np.float32)
    neighbors_np = np.asarray(neighbors)
    kern = np.asarray(kernel, dtype=np.float32)
    assert x.shape[1] == FEAT and kern.shape == (FEAT, U)
    assert neighbors_np.shape[1] == K

    in_maps, meta = _prep_inputs(x, neighbors_np, kern)
    key = (x.shape[0], meta["shard"], meta["C"])
    if key not in _CACHE:
        _CACHE[key] = build_kernel(x.shape[0], meta["shard"], meta["C"])
    nc = _CACHE[key]
    res = bass_utils.run_bass_kernel_spmd(
        nc, in_maps, core_ids=list(range(NUM_CORES))
    )
    return _unshard_output(res.results, meta)


# revision 8
# speedup vs baseline: 1.0374x; 1.0080x over previous
"""MedianConvolution (gnn message passing) — Trainium2 Bass kernel, 8 cores. v2

Computes: h = x @ kernel; msg = h[neighbors]; out = exact midpoint median
over the K=32 neighbor axis (ranks 15,16 of the sort).

v2 changes vs baseline:
  - h stored fp16 as row-PAIRS [nrows/2, 128]: one dma_gather per neighbor
    (idx = nbr>>1 fits int16; 256B blocks) instead of the lo/hi double
    gather -> gather DMA and Pool dispatch halved.
  - parity select on-chip: res = pair[0:64] ; copy_predicated overwrites
    with pair[64:128] where (nbr&1) using a host parity mask broadcast
    over units.
  - whole sort pipeline in fp16 (2x DVE throughput); x/kernel in bf16
    (halves phase-1 HBM traffic); PSUM->SBUF copies batched 8 col-tiles
    per ACT op.

Distribution: nodes sharded across 8 cores; every core computes the full
h = x @ kernel on-device (x replicated) and gathers its shard's rows.
"""
from contextlib import ExitStack

import numpy as np

import concourse.bass as bass
import concourse.tile as tile
from concourse import bacc, bass_utils, library_config, mybir
from concourse.tile_rust import add_dep_helper

F32 = mybir.dt.float32
F16 = mybir.dt.float16
BF16 = mybir.dt.bfloat16
I16 = mybir.dt.int16
U8 = mybir.dt.uint8
P = 128
U = 64  # units
K = 32  # neighbors
FEAT = 256
N_NODES = 50000
NUM_CORES = 8
CHUNK = 256  # shard nodes per chunk
NET_BUFS = 3
POOL_OPS = frozenset()

def chunk_plan(shard, C=CHUNK):
    """[(start_node, C_c, B_c)] covering >= shard nodes, 128-aligned; full
    C-sized chunks plus one minimal tail chunk (cuts pad-node sort waste)."""
    plan = []
    n = 0
    while shard - n >= C:
        plan.append((n, C, C // P))
        n += C
    if n < shard:
        rem = shard - n
        Cc = ((rem + P - 1) // P) * P
        plan.append((n, Cc, Cc // P))
    return plan


# Batcher odd-even mergesort(16) stages (verified via the 0-1 principle).
SORT16_STAGES = [
    dict(f=2, i=(0, 8, 1), r=(0, 1, 1), d=1, cp=[]),
    dict(f=4, i=(0, 4, 1), r=(0, 2, 1), d=2, cp=[]),
    dict(f=4, i=(0, 4, 1), r=(1, 2, 1), d=1, cp=[(0, 16, 4), (3, 16, 4)]),
    dict(f=8, i=(0, 2, 1), r=(0, 4, 1), d=4, cp=[]),
    dict(f=8, i=(0, 2, 1), r=(2, 4, 1), d=2,
         cp=[(0, 16, 8), (1, 16, 8), (6, 16, 8), (7, 16, 8)]),
    dict(f=8, i=(0, 2, 1), r=(1, 6, 2), d=1, cp=[(0, 16, 8), (7, 16, 8)]),
    dict(f=16, i=(0, 1, 1), r=(0, 8, 1), d=8, cp=[]),
    dict(f=16, i=(0, 1, 1), r=(4, 8, 1), d=4, cp=[(0, 4, 1), (12, 16, 1)]),
    dict(f=4, i=(0, 3, 1), r=(2, 4, 1), d=2, cp=[(0, 2, 1), (14, 16, 1)]),
    dict(f=2, i=(0, 7, 1), r=(1, 2, 1), d=1, cp=[(0, 16, 15)]),
]


def build_kernel(nrows, shard_nodes, C, num_cores=NUM_CORES, gemm_super=2048,
                 net_bufs=NET_BUFS, pool_ops=POOL_OPS, pool_every=0, lookahead=1,
                 pool_cols=0, srt_sets=2, XBUFS=3):
    assert nrows % 2 == 0
    HALF = nrows // 2
    plan = chunk_plan(shard_nodes, C)
    NCHUNK = len(plan)
    B = C // P
    NIDX = C * K
    IDXCOLS = NIDX // 16
    BU = B * U          # sort plane width (fp16 elems), full-size chunks
    KB = K * B          # gather planes per chunk, full-size chunks

    nc = bacc.Bacc(
        "TRN2",
        target_bir_lowering=False,
        debug=False,
        num_devices=num_cores,
    )

    xT = nc.dram_tensor("xT", [FEAT, nrows], F16, kind="ExternalInput").ap()
    wk = nc.dram_tensor("wk", [FEAT, U], F16, kind="ExternalInput").ap()
    idx = nc.dram_tensor("idx", [NCHUNK, P, IDXCOLS], I16, kind="ExternalInput").ap()
    par = nc.dram_tensor("par", [NCHUNK, P, KB], I16, kind="ExternalInput").ap()
    out = nc.dram_tensor("out", [NCHUNK, P, B * U], F32, kind="ExternalOutput").ap()
    # h rows fp16; gathered as row-pairs [HALF, 128]
    h = nc.dram_tensor("h", [nrows, U], F16, kind="Internal").ap()
    h_pairs = h.rearrange("(hh two) u -> hh (two u)", two=2)

    with tile.TileContext(nc) as tc:
        with ExitStack() as ctx:
            # ---------------- phase 1: GEMM ----------------
            ctx1 = ctx.enter_context(ExitStack())
            g_x = ctx1.enter_context(tc.tile_pool(name="g_x", bufs=XBUFS))
            g_w = ctx1.enter_context(tc.tile_pool(name="g_w", bufs=1))
            g_h = ctx1.enter_context(tc.tile_pool(name="g_h", bufs=2))
            g_ps = ctx1.enter_context(tc.tile_pool(name="g_ps", bufs=2, space="PSUM"))

            wkt = g_w.tile([P, 2 * U], F16)
            nc.sync.dma_start(wkt[:, 0:U], wk[0:P, :])
            nc.sync.dma_start(wkt[:, U : 2 * U], wk[P : 2 * P, :])

            h_writes = []
            S = gemm_super
            n_super = (nrows + S - 1) // S
            TPG = 8  # col-tiles per PSUM bank (8*64 = 512 fp32 = 2KB)
            for s in range(n_super):
                n0 = s * S
                ncnt = min(S, nrows - n0)
                ntiles = (ncnt + P - 1) // P
                xt0 = g_x.tile([P, S], F16, tag="xt0")
                xt1 = g_x.tile([P, S], F16, tag="xt1")
                nc.sync.dma_start(xt0[:, 0:ncnt], xT[0:P, n0 : n0 + ncnt])
                nc.sync.dma_start(xt1[:, 0:ncnt], xT[P : 2 * P, n0 : n0 + ncnt])
                hb = g_h.tile([P, (S // P) * U], F16, tag="hb")
                for tg in range(0, ntiles, TPG):
                    tn = min(TPG, ntiles - tg)
                    ps = g_ps.tile([P, TPG * U], F32)
                    for t in range(tg, tg + tn):
                        c0 = t * P
                        cw = min(P, ncnt - c0)
                        pslice = ps[0:cw, (t - tg) * U : (t - tg + 1) * U]
                        nc.tensor.matmul(
                            pslice, xt0[:, c0 : c0 + cw], wkt[:, 0:U],
                            start=True, stop=False,
                        )
                        nc.tensor.matmul(
                            pslice, xt1[:, c0 : c0 + cw], wkt[:, U : 2 * U],
                            start=False, stop=True,
                        )
                    # one batched fp32->fp16 copy per PSUM bank
                    nc.scalar.copy(
                        hb[:, tg * U : (tg + tn) * U], ps[:, 0 : tn * U]
                    )
                hb3 = hb[:].rearrange("p (t u) -> p t u", u=U)
                # write h rows [n0, n0+ncnt) (full 128-row tiles coalesced)
                lim0, lim1 = n0, n0 + ncnt
                ta = 0
                tb = (lim1 - n0) // P
                segs = []
                if tb > ta:
                    segs.append((n0, n0 + tb * P))
                if n0 + tb * P < lim1:
                    segs.append((n0 + tb * P, lim1))
                for r0, r1 in segs:
                    nt = (r1 - r0) // P
                    if nt >= 1 and (r0 - n0) % P == 0:
                        tt = (r0 - n0) // P
                        dr = h[r0:r1, :].rearrange("(o p) u -> p o u", p=P)
                        srcv = hb3[:, tt : tt + nt, :]
                    else:
                        tt = (r0 - n0) // P
                        p0 = r0 - (n0 + tt * P)
                        p1 = r1 - (n0 + tt * P)
                        dr = h[r0:r1, :].rearrange("(o p) u -> p o u", p=p1 - p0)
                        srcv = hb3[p0:p1, tt : tt + 1, :]
                    h_writes.append(nc.sync.dma_start(dr, srcv))

            # ---------------- phase 2: gather + select + median ----------------
            ctx1.close()
            g_net = ctx.enter_context(tc.tile_pool(name="g_net", bufs=net_bufs))
            g_srt = ctx.enter_context(tc.tile_pool(name="g_srt", bufs=2 * srt_sets))
            g_idx = ctx.enter_context(tc.tile_pool(name="g_idx", bufs=2))
            g_out = ctx.enter_context(tc.tile_pool(name="g_out", bufs=2))
            g_big = ctx.enter_context(tc.tile_pool(name="g_big", bufs=1))

            nc.gpsimd.load_library(library_config.mlp)
            med_all = g_big.tile([P, NCHUNK * B * U], F32, tag="medall")
            n_g = 0
            # per-call index count capped by the SWDGE ring
            KG = max(1, 1792 // C)
            kgroups = []
            k0 = 0
            while k0 < K:
                kgroups.append((k0, min(K, k0 + KG)))
                k0 += KG

            gat_tiles = {}

            def emit_tt(out, in0, in1, op):
                if pool_cols:
                    cs = out.shape[-1] - pool_cols
                    sl_d = (Ellipsis, slice(0, cs))
                    sl_p = (Ellipsis, slice(cs, out.shape[-1]))
                    nc.vector.tensor_tensor(
                        out=out[sl_d], in0=in0[sl_d], in1=in1[sl_d], op=op)
                    nc.gpsimd.tensor_tensor(
                        out=out[sl_p], in0=in0[sl_p], in1=in1[sl_p], op=op)
                else:
                    nc.vector.tensor_tensor(out=out, in0=in0, in1=in1, op=op)

            def emit_gather(c):
                nonlocal n_g
                if c >= NCHUNK or c in gat_tiles:
                    return
                _, Cc, Bc = plan[c]
                KBc = K * Bc
                IDXCOLSc = Cc * K // 16
                KGc = max(1, 1792 // Cc)
                kgroups_c = [(k0, min(K, k0 + KGc)) for k0 in range(0, K, KGc)]
                ia = g_idx.tile([P, IDXCOLS], I16, tag="ia")
                nc.sync.dma_start(ia[:, 0:IDXCOLSc], idx[c, :, 0:IDXCOLSc])
                gat = g_net.tile([P, KB * 2 * U], F16, tag="gat")
                gat3 = gat[:, 0 : KBc * 2 * U].rearrange("p (j e) -> p j e", e=2 * U)
                for ka, kb in kgroups_c:
                    nidx = Cc * (kb - ka)
                    g = nc.gpsimd.dma_gather(
                        gat3[:, ka * Bc : kb * Bc, :],
                        h_pairs,
                        ia[:, ka * Cc // 16 : kb * Cc // 16],
                        nidx,
                        nidx,
                        2 * U,
                        single_packet=False,
                    )
                    if n_g == 0:
                        for w in h_writes:
                            add_dep_helper(
                                g.ins, w.ins,
                                reason="gather waits for h DRAM writes",
                            )
                    n_g += 1
                gat_tiles[c] = gat

            for c in range(lookahead):
                emit_gather(c)

            pend = None  # (ra_t, rb_t) of a pair's first chunk awaiting finale
            for c in range(NCHUNK):
                on_pool = pool_every and (c % pool_every == pool_every - 1)

                def VE(tag, _on_pool=on_pool):
                    return nc.gpsimd if (_on_pool or tag in pool_ops) else nc.vector

                _, Cc, Bc = plan[c]
                BUc = Bc * U
                KBc = K * Bc
                emit_gather(c + lookahead)
                gat = gat_tiles.pop(c)
                gat3 = gat[:, 0 : KBc * 2 * U].rearrange("p (j e) -> p j e", e=2 * U)
                pa = g_idx.tile([P, KB], I16, tag="pa")
                nc.sync.dma_start(pa[:, 0:KBc], par[c, :, 0:KBc])
                # parity select in place: overwrite gat's E columns with O
                # where the neighbor is odd; stage 1 then reads the selected
                # (strided) columns directly.
                can_pair = (
                    pend is None
                    and Bc == B
                    and c + 1 < NCHUNK
                    and plan[c + 1][2] == B
                )
                if pend is not None:
                    ra_t, rb_t = pend
                    half = K * BU
                else:
                    ra_t = g_srt.tile([P, 2 * K * BU], F16, tag="ra")
                    rb_t = g_srt.tile([P, 2 * K * BU], F16, tag="rb")
                    half = 0
                ra = ra_t[:, half : half + K * BUc]
                rb = rb_t[:, half : half + K * BUc]
                nc.vector.copy_predicated(
                    out=gat3[:, :, 0:U],
                    mask=pa[:, 0:KBc].rearrange("p (j o) -> p j o", o=1)
                    .to_broadcast([P, KBc, U]),
                    data=gat3[:, :, U : 2 * U],
                )

                # Batcher network; stage 1 reads gat (strided), writes ra;
                # stages 2+ ping-pong ra <-> rb
                gsel = gat[:, 0 : KBc * 2 * U].rearrange(
                    "p (hi r b e) -> p hi r b e", hi=16, r=2, b=Bc, e=2 * U
                )
                src, dst = rb, ra
                for si, sp in enumerate(SORT16_STAGES):
                    eng = VE(f"s{si}")
                    f = sp["f"]
                    ni = 16 // f
                    i_full = sp["i"] == (0, ni, 1)
                    d = sp["d"]
                    di, dr = d // f, d % f
                    r_vals = list(range(*sp["r"]))
                    if r_vals[-1] + dr >= f:
                        assert all(rv + dr >= f for rv in r_vals), sp
                        di, dr = di + 1, dr - f
                    r_sl = slice(*sp["r"])
                    hi_r = slice(sp["r"][0] + dr, sp["r"][1] + dr, sp["r"][2])
                    if i_full and di == 0:
                        if si == 0:
                            vs = None
                            vd = dst[:, 0 : K * BUc].rearrange(
                                "p (hi r b u) -> p hi r b u", hi=16, r=2, b=Bc, u=U
                            )
                            lo_s = gsel[:, :, r_sl, :, 0:U]
                            hi_s = gsel[:, :, hi_r, :, 0:U]
                            emit_tt(vd[:, :, r_sl, :, :], lo_s, hi_s,
                                    mybir.AluOpType.min)
                            emit_tt(vd[:, :, hi_r, :, :], lo_s, hi_s,
                                    mybir.AluOpType.max)
                            src, dst = dst, src
                            continue
                        vs = src[:, 0 : K * BUc].rearrange(
                            "p (hi r bu) -> p hi r bu", r=f, bu=BUc)
                        vd = dst[:, 0 : K * BUc].rearrange(
                            "p (hi r bu) -> p hi r bu", r=f, bu=BUc)
                        lo_s = vs[:, :, r_sl, :]
                        hi_s = vs[:, :, hi_r, :]
                        emit_tt(vd[:, :, r_sl, :], lo_s, hi_s,
                                mybir.AluOpType.min)
                        emit_tt(vd[:, :, hi_r, :], lo_s, hi_s,
                                mybir.AluOpType.max)
                    else:
                        i_sl = slice(*sp["i"])
                        hi_i = slice(sp["i"][0] + di, sp["i"][1] + di, sp["i"][2])
                        vs = src[:, 0 : K * BUc].rearrange(
                            "p (hh i r bu) -> p hh i r bu", hh=2, i=ni, r=f, bu=BUc
                        )
                        vd = dst[:, 0 : K * BUc].rearrange(
                            "p (hh i r bu) -> p hh i r bu", hh=2, i=ni, r=f, bu=BUc
                        )
                        lo_s = vs[:, :, i_sl, r_sl, :]
                        hi_s = vs[:, :, hi_i, hi_r, :]
                        emit_tt(vd[:, :, i_sl, r_sl, :], lo_s, hi_s,
                                mybir.AluOpType.min)
                        emit_tt(vd[:, :, hi_i, hi_r, :], lo_s, hi_s,
                                mybir.AluOpType.max)
                    vks = src[:, 0 : K * BUc].rearrange(
                        "p (hh kk bu) -> p hh kk bu", hh=2, kk=16)
                    vkd = dst[:, 0 : K * BUc].rearrange(
                        "p (hh kk bu) -> p hh kk bu", hh=2, kk=16)
                    for cpsl in sp["cp"]:
                        ks = slice(*cpsl)
                        nc.scalar.copy(vkd[:, :, ks, :], vks[:, :, ks, :])
                    src, dst = dst, src

                # after 10 stages (even number of swaps from (rb, ra)) the
                # sorted halves live in rb, scratch in ra
                assert src is rb
                if can_pair:
                    pend = (ra_t, rb_t)
                    continue
                if pend is not None:
                    # batched finale for the pair (chunks c-1, c): one set of
                    # double-width antidiag + tree ops
                    pend = None
                    two = 2
                    vk = rb_t[:].rearrange(
                        "p (two k bu) -> p two k bu", two=two, k=K)
                    vo = ra_t[:].rearrange(
                        "p (two k bu) -> p two k bu", two=two, k=K)
                    A = vk[:, :, 0:16, :]
                    Brev = vk[:, :, 31:15:-1, :]
                    emit_tt(vo[:, :, 0:16, :], A, Brev, mybir.AluOpType.max)
                    emit_tt(vk[:, :, 0:16, :], A, Brev, mybir.AluOpType.min)
                    for buf, op in ((vk, mybir.AluOpType.max),
                                    (vo, mybir.AluOpType.min)):
                        w = 8
                        while w >= 1:
                            emit_tt(buf[:, :, 0:w, :], buf[:, :, 0:w, :],
                                    buf[:, :, w : 2 * w, :], op)
                            w //= 2
                    ms = med_all[:, (c - 1) * BU : (c + 1) * BU]
                    nc.vector.tensor_tensor(
                        out=ms.rearrange("p (two bu) -> p two bu", two=two),
                        in0=vk[:, :, 0, :], in1=vo[:, :, 0, :],
                        op=mybir.AluOpType.add,
                    )
                    nc.scalar.mul(ms, ms, 0.5)
                    nc.sync.dma_start(out[c - 1, :, 0:BU], ms[:, 0:BU])
                    nc.sync.dma_start(out[c, :, 0:BU], ms[:, BU : 2 * BU])
                else:
                    # solo finale (tail chunk)
                    vk = src[:, 0 : K * BUc].rearrange("p (k bu) -> p k bu", k=K)
                    vo = dst[:, 0 : K * BUc].rearrange("p (k bu) -> p k bu", k=K)
                    A = vk[:, 0:16, :]
                    Brev = vk[:, 31:15:-1, :]
                    emit_tt(vo[:, 0:16, :], A, Brev, mybir.AluOpType.max)
                    emit_tt(vk[:, 0:16, :], A, Brev, mybir.AluOpType.min)
                    for buf, op in ((vk, mybir.AluOpType.max),
                                    (vo, mybir.AluOpType.min)):
                        w = 8
                        while w >= 1:
                            emit_tt(buf[:, 0:w, :], buf[:, 0:w, :],
                                    buf[:, w : 2 * w, :], op)
                            w //= 2
                    ms = med_all[:, c * BU : c * BU + BUc]
                    nc.vector.tensor_tensor(
                        out=ms, in0=vk[:, 0, :], in1=vo[:, 0, :],
                        op=mybir.AluOpType.add,
                    )
                    nc.scalar.mul(ms, ms, 0.5)
                    nc.sync.dma_start(out[c, :, 0:BUc], ms)

    nc.compile()
    return nc


def _prep_inputs(x, neighbors, kern, num_cores=NUM_CORES, C=CHUNK):
    nrows = x.shape[0]
    total = neighbors.shape[0]
    shard = (total + num_cores - 1) // num_cores
    plan = chunk_plan(shard, C)
    NCHUNK = len(plan)
    shard_pad = plan[-1][0] + plan[-1][1]
    B = C // P
    NIDX = C * K
    IDXCOLS = NIDX // 16
    KB = K * B

    xT = np.ascontiguousarray(x.T).astype(np.float16)
    wk = np.ascontiguousarray(kern).astype(np.float16)

    in_maps = []
    for core in range(num_cores):
        n0 = core * shard
        nbr = np.zeros((shard_pad, K), dtype=np.int64)
        real = min(shard, total - n0)
        nbr[:real] = neighbors[n0 : n0 + real]
        idxarr = np.zeros((NCHUNK, P, IDXCOLS), dtype=np.int16)
        pararr = np.zeros((NCHUNK, P, KB), dtype=np.int16)
        for c, (n0c, Cc, Bc) in enumerate(plan):
            IDXCOLSc = Cc * K // 16
            KBc = K * Bc
            nb3 = nbr[n0c : n0c + Cc].reshape(Bc, P, K)
            v = nb3.transpose(2, 0, 1).reshape(-1)  # i = ((k*Bc + b)*128 + p)
            pair = (v >> 1).astype(np.int16)
            # logical index i lives at [i%16, i//16]; replicated to all
            # eight 16-partition groups (Q7 core pairs read their own)
            idxarr[c, :, 0:IDXCOLSc] = np.tile(
                pair.reshape(IDXCOLSc, 16).T, (P // 16, 1)
            )
            pararr[c, :, 0:KBc] = (v & 1).astype(np.int16).reshape(KBc, P).T
        in_maps.append({"xT": xT, "wk": wk, "idx": idxarr, "par": pararr})
    meta = dict(shard=shard, shard_pad=shard_pad, NCHUNK=NCHUNK, C=C, total=total,
                plan=plan)
    return in_maps, meta


def _unshard_output(results, meta, num_cores=NUM_CORES):
    outs = []
    for core in range(num_cores):
        o = results[core]["out"]  # [NCHUNK, P, B*U]
        full = np.empty((meta["shard_pad"], U), dtype=o.dtype)
        for c, (n0c, Cc, Bc) in enumerate(meta["plan"]):
            oc = o[c, :, 0 : Bc * U].reshape(P, Bc, U)
            full[n0c : n0c + Cc] = oc.transpose(1, 0, 2).reshape(Cc, U)
        outs.append(full[: meta["shard"]])
    return np.concatenate(outs, axis=0)[: meta["total"]]


_CACHE = {}


def kernel(x, neighbors, kernel):
    """Full inputs in, full output out. Shards nodes across 8 NeuronCores."""
    x = np.asarray(x, dtype=np.float32)
    neighbors_np = np.asarray(neighbors)
    kern = np.asarray(kernel, dtype=np.float32)
    assert x.shape[1] == FEAT and kern.shape == (FEAT, U)
    assert neighbors_np.shape[1] == K

    in_maps, meta = _prep_inputs(x, neighbors_np, kern)
    key = (x.shape[0], meta["shard"], meta["C"])
    if key not in _CACHE:
        _CACHE[key] = build_kernel(x.shape[0], meta["shard"], meta["C"])
    nc = _CACHE[key]
    res = bass_utils.run_bass_kernel_spmd(
        nc, in_maps, core_ids=list(range(NUM_CORES))
    )
    return _unshard_output(res.results, meta)
